# revision 20
# baseline (speedup 1.0000x reference)
"""Trainium2 Bass kernel for a batched GAT layer (BGATLayer).

Reference computation (per batch b of B=16, N=1024 nodes, F=512 features):
    h   = x @ W                                   # [N, F]
    s1  = h @ a1 ; s2 = h @ a2                    # [N]
    e   = leakyrelu(s1[:,None] + s2[None,:], 0.2) # [N, N]
    att = softmax(e, axis=1)                      # row softmax
    out = elu(att @ h + beta * h)                 # [N, F]

Sharding: batch B=16 split across 8 NeuronCores (2 batches/core, data
parallel); W/a/beta replicated.

v2 design (v1 measured 147us; PE busy 107us incl ~40us at HAM half
clock from C-phase starvation, plus a 19us epilogue tail):
  * All matmul operands are bf16 (tolerance is 2e-2; bf16 adds ~4e-3).
    Streaming rate is the same 1 cyc/row as f32r@512, but LDWEIGHTS is
    ~2x faster and SBUF footprint halves.
  * uT tiles are computed with ZERO PE work: uT[j][p,i] =
    exp(prelu(s1[i] + s2[j*128+p])).  s1 enters as a [128, N]
    partition-broadcast tile (SBUF->SBUF DMA), s2 as a per-partition
    bias column (ACT activation bias= accepts a [128,1] AP; DVE path
    uses tensor_scalar with an AP scalar).  v1 computed each z tile as
    a K=2 PE matmul that ping-ponged with ACT and starved the PE into
    the HAM's k=4/8 duty-cycle downclock.
  * PE stream is matmul-only and back-to-back:
      warmup T0 S0 B0 T1 S1 B1 R0 DE0(p) R1 DE1(p)
    so the activity monitor keeps the clock at max.
  * rowsum still via ones-stationary matmuls (cheap: 2-row stationary),
    reciprocal through the DRAM row->column roundtrip.
  * epilogue per tile: v = p*recip + h (DVE stt, mixed f32/bf16),
    m = min(v,0) (DVE), em = exp(m) (ACT), out = max(em-1, v) (DVE),
    trailing the p matmuls tile-by-tile instead of bunching at the end.
"""

import sys

sys.path.insert(0, "/opt/trn_rl_repo")

from contextlib import ExitStack

import numpy as np

import concourse.bacc as bacc
import concourse.bass as bass
import concourse.mybir as mybir
from concourse.bass_utils import run_bass_kernel_spmd
from concourse.masks import make_identity
from concourse.tile import TileContext

P = 128
N_NODES = 1024
F = 512
B_TOTAL = 16
N_CORES = 8
B_PER_CORE = B_TOTAL // N_CORES
NK = F // P  # 4 contraction chunks for x @ W
NN = N_NODES // P  # 8 node chunks
ALPHA = 0.2

F32 = mybir.dt.float32
F32R = mybir.dt.float32r
BF16 = mybir.dt.bfloat16
AL = mybir.AluOpType
AF = mybir.ActivationFunctionType


def _r(ap):
    """float32r view of an fp32 AP (PE reduced-precision matmul mode)."""
    return ap.bitcast(F32R)


def build_nc(beta_val: float = 1.0) -> bass.Bass:
    nc = bacc.Bacc("TRN2")
    x_d = nc.dram_tensor("x", [B_PER_CORE, N_NODES, F], F32, kind="ExternalInput")
    w_d = nc.dram_tensor("W", [F, F], F32, kind="ExternalInput")
    a_d = nc.dram_tensor("a", [2 * F, 1], F32, kind="ExternalInput")
    beta_d = nc.dram_tensor("beta", [1], F32, kind="ExternalInput")
    out_d = nc.dram_tensor("out", [B_PER_CORE, N_NODES, F], F32, kind="ExternalOutput")
    # scratch for row->per-partition-column roundtrips
    r_d = nc.dram_tensor("r_scratch", [B_PER_CORE, N_NODES], F32)
    s_d = nc.dram_tensor("s_scratch", [B_PER_CORE, 2, N_NODES], F32)
    e_d = nc.dram_tensor("e_scratch", [B_PER_CORE, 2, N_NODES], F32)

    with TileContext(nc) as tc, ExitStack() as ctx:
        # ---------------- pools ----------------
        singles = ctx.enter_context(tc.tile_pool(name="singles", bufs=1))
        xin = ctx.enter_context(tc.tile_pool(name="xin", bufs=16))
        xtp = ctx.enter_context(tc.tile_pool(name="xtp", bufs=2))  # xT bf16
        hpool = ctx.enter_context(tc.tile_pool(name="hpool", bufs=16))
        spool = ctx.enter_context(tc.tile_pool(name="spool", bufs=2))
        utp = ctx.enter_context(tc.tile_pool(name="utp", bufs=16))
        tpool = ctx.enter_context(tc.tile_pool(name="tpool", bufs=3))
        qp = ctx.enter_context(tc.tile_pool(name="qp", bufs=2))
        epool = ctx.enter_context(tc.tile_pool(name="epool", bufs=4))
        # PSUM: ps_tr 2x[128,512](2 banks) ps_mm 4x[128,512](4) ps_s 1x[2,1024](2)
        ps_tr = ctx.enter_context(tc.tile_pool(name="ps_tr", bufs=2, space="PSUM"))
        ps_mm = ctx.enter_context(tc.tile_pool(name="ps_mm", bufs=4, space="PSUM"))
        ps_s = ctx.enter_context(tc.tile_pool(name="ps_s", bufs=1, space="PSUM"))

        # ---------------- prologue ----------------
        identf = singles.tile([P, P], F32, tag="identf")
        make_identity(nc, identf)
        ident = singles.tile([P, P], F32, tag="ident")
        nc.scalar.copy(out=_r(ident), in_=identf)

        ones2b = singles.tile([P, 2], BF16, tag="ones2b")
        nc.gpsimd.memset(ones2b, 1.0)

        a_flat = a_d.rearrange("f one -> (f one)")
        a1b = singles.tile([P, F], F32, tag="a1b")
        a2b = singles.tile([P, F], F32, tag="a2b")
        beta_sb = singles.tile([1, 1], F32, tag="beta_sb")
        w_sb = []
        wb = []
        for k in range(NK):
            wk = singles.tile([P, F], F32, tag=f"w_sb{k}")
            w_sb.append(wk)
            wbk = singles.tile([P, F], BF16, tag=f"wb{k}")
            wb.append(wbk)
        w12b = singles.tile([P, 2 * NK], BF16, tag="w12b")

        def load_weights():
            nc.sync.dma_start(out=a1b, in_=a_flat[0:F].partition_broadcast(P))
            nc.sync.dma_start(out=a2b, in_=a_flat[F : 2 * F].partition_broadcast(P))
            # beta lands in SBUF only to keep the input bound (value baked)
            nc.sync.dma_start(out=beta_sb, in_=beta_d[0:1].unsqueeze(0))
            for k in range(NK):
                nc.sync.dma_start(out=w_sb[k], in_=w_d[k * P : (k + 1) * P, :])
                # bf16 copy of W for the h matmul (moving operand)
                nc.scalar.copy(out=wb[k], in_=w_sb[k])
                w12f = qp.tile([P, 2], F32, tag="w12f")
                prod = qp.tile([P, F], F32, tag="wa_prod")
                for j, ab in enumerate((a1b, a2b)):
                    nc.vector.tensor_tensor(
                        out=prod, in0=w_sb[k], in1=ab, op=AL.mult
                    )
                    nc.vector.reduce_sum(
                        out=w12f[:, j : j + 1], in_=prod, axis=mybir.AxisListType.X
                    )
                nc.scalar.copy(out=w12b[:, 2 * k : 2 * k + 2], in_=w12f)

        # ---------------- PE warm-up ----------------
        # hold the activity monitor busy during the initial DMA window so
        # real matmuls start at the max clock
        for _ in range(4):
            wp = ps_tr.tile([P, F], F32, tag="ps_tr")
            nc.tensor.transpose(_r(wp[:, 0:P]), _r(ident), _r(ident))
            nc.tensor.transpose(_r(wp[:, P : 2 * P]), _r(ident), _r(ident))

        # ---------------- per-batch state ----------------
        xt_alls = {}
        h_sbs = {}
        uts = {}
        rcols = {}
        e1bs = {}
        e1abs = {}
        e2cols = {}
        e2acols = {}
        x_tiles = {}

        def phase_A_dma(b):  # issue all x loads for this batch
            x_tiles[b] = []
            for n in range(NN):
                x_t = xin.tile([P, F], F32, tag="x_t")
                nc.sync.dma_start(out=_r(x_t), in_=_r(x_d[b, n * P : (n + 1) * P, :]))
                x_tiles[b].append(x_t)

        def phase_T(b):  # transpose x into bf16 xT
            xt_all = xtp.tile([P, NK * N_NODES], BF16, tag="xt_all")
            xt_alls[b] = xt_all
            for n in range(NN):
                x_t = x_tiles[b][n]
                xp = ps_tr.tile([P, F], F32, tag="ps_tr")
                for k in range(NK):
                    nc.tensor.transpose(
                        _r(xp[:, k * P : (k + 1) * P]),
                        _r(x_t[:, k * P : (k + 1) * P]),
                        _r(ident),
                    )
                dst = xt_all.rearrange("p (k c) -> p k c", k=NK)[
                    :, :, n * P : (n + 1) * P
                ]
                src = xp.rearrange("p (k c) -> p k c", k=NK)
                nc.vector.tensor_copy(out=dst, in_=src)

        def phase_S(b):
            # s rows; then u's factorization u = max(e^{s1}e^{s2},
            # e^{.2 s1}e^{.2 s2}) needs only TINY exps: on [128,8] column
            # forms (vs 16 NxN-sized ACT passes).  E1/E1a go back out as
            # bf16 rows and return as partition-broadcast [128, N] tiles.
            xt_all = xt_alls[b]
            s_ps = ps_s.tile([2, N_NODES], F32, tag="ps_s")
            for k in range(NK):
                for hh in range(2):
                    nc.tensor.matmul(
                        s_ps[:, hh * F : (hh + 1) * F],
                        lhsT=w12b[:, 2 * k : 2 * k + 2],
                        rhs=xt_all[:, k * N_NODES + hh * F : k * N_NODES + (hh + 1) * F],
                        start=(k == 0),
                        stop=(k == NK - 1),
                    )
            s_sb = spool.tile([2, N_NODES], F32, tag="s_sb")
            nc.vector.tensor_copy(out=s_sb, in_=s_ps)
            nc.sync.dma_start(out=s_d[b], in_=s_sb)
            # both s rows as per-partition columns: [128, 16] = s1 | s2
            s12col = spool.tile([P, 2 * NN], F32, tag="s12col")
            nc.sync.dma_start(
                out=s12col, in_=s_d[b].rearrange("t (n p) -> p (t n)", p=P)
            )
            e1col = spool.tile([P, NN], F32, tag="e1col")
            nc.scalar.activation(out=e1col, in_=s12col[:, 0:NN], func=AF.Exp)
            e1acol = spool.tile([P, NN], F32, tag="e1acol")
            nc.scalar.activation(
                out=e1acol, in_=s12col[:, 0:NN], func=AF.Exp, scale=ALPHA
            )
            e2col = spool.tile([P, NN], F32, tag="e2col")
            nc.scalar.activation(out=e2col, in_=s12col[:, NN : 2 * NN], func=AF.Exp)
            e2cols[b] = e2col
            e2acol = spool.tile([P, NN], F32, tag="e2acol")
            nc.scalar.activation(
                out=e2acol, in_=s12col[:, NN : 2 * NN], func=AF.Exp, scale=ALPHA
            )
            e2acols[b] = e2acol
            # E1 rows out to DRAM (column->row scatter), back as broadcasts
            nc.sync.dma_start(
                out=e_d[b, 0].rearrange("(n p) -> p n", p=P), in_=e1col
            )
            nc.sync.dma_start(
                out=e_d[b, 1].rearrange("(n p) -> p n", p=P), in_=e1acol
            )
            e1bf = spool.tile([P, N_NODES], F32, tag="e1bf")
            nc.sync.dma_start(out=e1bf, in_=e_d[b, 0].partition_broadcast(P))
            e1abf = spool.tile([P, N_NODES], F32, tag="e1abf")
            nc.sync.dma_start(out=e1abf, in_=e_d[b, 1].partition_broadcast(P))
            # bf16 working copies (enables DVE 2x_1p/4x_2p in the C phase)
            e1b = spool.tile([P, N_NODES], BF16, tag="e1b")
            e1bs[b] = e1b
            nc.vector.tensor_copy(out=e1b, in_=e1bf)
            e1ab = spool.tile([P, N_NODES], BF16, tag="e1ab")
            e1abs[b] = e1ab
            nc.vector.tensor_copy(out=e1ab, in_=e1abf)

        def emit_C_tile(b, j):
            # uT[j][p, i] = max(E1[i]E2[jp], E1a[i]E2a[jp]) -- 3 DVE ops in
            # 2x/4x fast mode (bf16 packed SBUF operands; per-partition
            # scalars are dtype-exempt), zero ACT, zero PE.
            t1 = tpool.tile([P, N_NODES], BF16, tag="t1")
            nc.vector.tensor_scalar(
                out=t1, in0=e1bs[b], scalar1=e2cols[b][:, j : j + 1], scalar2=None,
                op0=AL.mult,
            )
            t2 = tpool.tile([P, N_NODES], BF16, tag="t2")
            nc.vector.tensor_scalar(
                out=t2, in0=e1abs[b], scalar1=e2acols[b][:, j : j + 1], scalar2=None,
                op0=AL.mult,
            )
            u = utp.tile([P, N_NODES], BF16, tag="ut")
            nc.vector.tensor_tensor(out=u, in0=t1, in1=t2, op=AL.max)
            uts[b][j] = u

        def phase_C(b):
            uts[b] = [None] * NN
            for j in range(NN):
                emit_C_tile(b, j)

        def phase_B(b):  # h = x @ W  (bf16 out for the p matmul + epilogue)
            xt_all = xt_alls[b]
            h_sbs[b] = []
            for n in range(NN):
                h_ps = ps_mm.tile([P, F], F32, tag="ps_mm")
                for k in range(NK):
                    nc.tensor.matmul(
                        h_ps,
                        lhsT=xt_all[:, k * N_NODES + n * P : k * N_NODES + (n + 1) * P],
                        rhs=wb[k],
                        start=(k == 0),
                        stop=(k == NK - 1),
                    )
                ht = hpool.tile([P, F], BF16, tag="h_sb")
                nc.scalar.copy(out=ht, in_=h_ps)
                h_sbs[b].append(ht)

        def phase_R(b):  # rowsum -> reciprocal columns
            ut = uts[b]
            rs_ps = ps_s.tile([2, N_NODES], F32, tag="ps_s")
            for j in range(NN):
                for hh in range(2):
                    nc.tensor.matmul(
                        rs_ps[:, hh * F : (hh + 1) * F],
                        lhsT=ones2b,
                        rhs=ut[j][:, hh * F : (hh + 1) * F],
                        start=(j == 0),
                        stop=(j == NN - 1),
                    )
            rrow = spool.tile([1, N_NODES], F32, tag="rrow")
            nc.vector.tensor_copy(out=rrow, in_=rs_ps[0:1, :])
            nc.sync.dma_start(out=r_d[b].unsqueeze(0), in_=rrow)
            rcraw = spool.tile([P, NN], F32, tag="rcraw")
            nc.sync.dma_start(out=rcraw, in_=r_d[b].rearrange("(n p) -> p n", p=P))
            rcol = spool.tile([P, NN], F32, tag="rcol")
            rcols[b] = rcol
            nc.vector.reciprocal(out=rcol, in_=rcraw)

        def emit_DE_tile(b, n):  # p[n] = u @ h + fused ELU epilogue
            ut, h_sb, rcol = uts[b], h_sbs[b], rcols[b]
            p_ps = ps_mm.tile([P, F], F32, tag="ps_mm")
            for j in range(NN):
                nc.tensor.matmul(
                    p_ps,
                    lhsT=ut[j][:, n * P : (n + 1) * P],
                    rhs=h_sb[j],
                    start=(j == 0),
                    stop=(j == NN - 1),
                )
            hin = h_sb[n]
            if beta_val != 1.0:
                hb = epool.tile([P, F], F32, tag="hb")
                nc.vector.tensor_scalar_mul(hb, hin, float(beta_val))
                hin = hb
            v = epool.tile([P, F], BF16, tag="v")
            # v = p * (1/rowsum) + beta*h (the one unavoidable 1x stt)
            nc.vector.scalar_tensor_tensor(
                out=v, in0=p_ps, scalar=rcol[:, n : n + 1], in1=hin,
                op0=AL.mult, op1=AL.add,
            )
            # elu(v) = max(exp(min(v,0)) - 1, v); min(v,0) = -relu(-v) on ACT
            rl = epool.tile([P, F], BF16, tag="rl")
            nc.scalar.activation(out=rl, in_=v, func=AF.Relu, scale=-1.0)
            em = epool.tile([P, F], BF16, tag="em")
            nc.scalar.activation(out=em, in_=rl, func=AF.Exp, scale=-1.0)
            o1 = epool.tile([P, F], BF16, tag="o1")
            nc.vector.tensor_scalar(
                out=o1, in0=em, scalar1=-1.0, scalar2=None, op0=AL.add
            )
            o = epool.tile([P, F], F32, tag="o")
            nc.vector.tensor_tensor(out=o, in0=o1, in1=v, op=AL.max)
            nc.sync.dma_start(out=out_d[b, n * P : (n + 1) * P, :], in_=o)

        # ------------- software-pipelined emission -------------
        # PE order: warmup T0 S0 B0 T1 S1 B1 R0 DE0 R1 DE1 -- back-to-back
        # matmuls, never paced by ACT/DVE.  C phases are pure ACT/DVE and
        # run concurrently (C0 under B0/T1, C1 under B1/DE0).
        phase_A_dma(0)
        load_weights()
        phase_T(0)
        phase_S(0)
        phase_A_dma(1)
        phase_B(0)
        phase_T(1)
        phase_C(0)
        phase_S(1)
        phase_B(1)
        phase_C(1)
        phase_R(0)
        for n in range(5):
            emit_DE_tile(0, n)
        phase_R(1)
        for n in range(5, NN):
            emit_DE_tile(0, n)
        for n in range(NN):
            emit_DE_tile(1, n)

    nc.finalize()
    return nc


_NC_CACHE = {}


def _get_nc(beta_val: float) -> bass.Bass:
    key = float(beta_val)
    if key not in _NC_CACHE:
        _NC_CACHE[key] = build_nc(beta_val=key)
    return _NC_CACHE[key]


def kernel(x, W, a, beta, _trace=False, _mm_fp32=False):
    x = np.ascontiguousarray(x, dtype=np.float32)
    W = np.ascontiguousarray(W, dtype=np.float32)
    a = np.ascontiguousarray(a, dtype=np.float32)
    beta = np.ascontiguousarray(beta, dtype=np.float32)

    nc = _get_nc(float(beta.reshape(-1)[0]))
    in_maps = [
        {
            "x": x[c * B_PER_CORE : (c + 1) * B_PER_CORE],
            "W": W,
            "a": a,
            "beta": beta,
        }
        for c in range(N_CORES)
    ]
    res = run_bass_kernel_spmd(nc, in_maps, core_ids=list(range(N_CORES)), trace=_trace)
    out = np.concatenate(
        [np.asarray(r["out"]).astype(np.float32) for r in res.results], axis=0
    )
    if _trace:
        kernel.last_exec_time_ns = res.exec_time_ns
        kernel.last_results = res
    return out


if __name__ == "__main__":
    rng = np.random.default_rng(0)
    x = rng.standard_normal((B_TOTAL, N_NODES, F), dtype=np.float32)
    W = rng.standard_normal((F, F), dtype=np.float32) * 0.05
    a = rng.standard_normal((2 * F, 1), dtype=np.float32) * 0.05
    beta = np.ones((1,), dtype=np.float32)
    out = kernel(x, W, a, beta)
    print("out", out.shape, out.dtype)


# revision 31
# speedup vs baseline: 1.2419x; 1.2419x over previous
"""Trainium2 Bass kernel for a batched GAT layer (BGATLayer).

Reference computation (per batch b of B=16, N=1024 nodes, F=512 features):
    h   = x @ W                                   # [N, F]
    s1  = h @ a1 ; s2 = h @ a2                    # [N]
    e   = leakyrelu(s1[:,None] + s2[None,:], 0.2) # [N, N]
    att = softmax(e, axis=1)                      # row softmax
    out = elu(att @ h + beta * h)                 # [N, F]

Sharding: batch B=16 split across 8 NeuronCores (2 batches/core, data
parallel); W/a/beta replicated.

v2 design (v1 measured 147us; PE busy 107us incl ~40us at HAM half
clock from C-phase starvation, plus a 19us epilogue tail):
  * All matmul operands are bf16 (tolerance is 2e-2; bf16 adds ~4e-3).
    Streaming rate is the same 1 cyc/row as f32r@512, but LDWEIGHTS is
    ~2x faster and SBUF footprint halves.
  * uT tiles are computed with ZERO PE work: uT[j][p,i] =
    exp(prelu(s1[i] + s2[j*128+p])).  s1 enters as a [128, N]
    partition-broadcast tile (SBUF->SBUF DMA), s2 as a per-partition
    bias column (ACT activation bias= accepts a [128,1] AP; DVE path
    uses tensor_scalar with an AP scalar).  v1 computed each z tile as
    a K=2 PE matmul that ping-ponged with ACT and starved the PE into
    the HAM's k=4/8 duty-cycle downclock.
  * PE stream is matmul-only and back-to-back:
      warmup T0 S0 B0 T1 S1 B1 R0 DE0(p) R1 DE1(p)
    so the activity monitor keeps the clock at max.
  * rowsum still via ones-stationary matmuls (cheap: 2-row stationary),
    reciprocal through the DRAM row->column roundtrip.
  * epilogue per tile: v = p*recip + h (DVE stt, mixed f32/bf16),
    m = min(v,0) (DVE), em = exp(m) (ACT), out = max(em-1, v) (DVE),
    trailing the p matmuls tile-by-tile instead of bunching at the end.
"""

import sys

sys.path.insert(0, "/opt/trn_rl_repo")

from contextlib import ExitStack

import numpy as np

import concourse.bacc as bacc
import concourse.bass as bass
import concourse.mybir as mybir
from concourse.bass_utils import run_bass_kernel_spmd
from concourse.masks import make_identity
from concourse.tile import TileContext

P = 128
N_NODES = 1024
F = 512
B_TOTAL = 16
N_CORES = 8
B_PER_CORE = B_TOTAL // N_CORES
NK = F // P  # 4 contraction chunks for x @ W
NN = N_NODES // P  # 8 node chunks
ALPHA = 0.2

F32 = mybir.dt.float32
F32R = mybir.dt.float32r
BF16 = mybir.dt.bfloat16
AL = mybir.AluOpType
AF = mybir.ActivationFunctionType


def _r(ap):
    """float32r view of an fp32 AP (PE reduced-precision matmul mode)."""
    return ap.bitcast(F32R)


def build_nc(beta_val: float = 1.0) -> bass.Bass:
    nc = bacc.Bacc("TRN2")
    x_d = nc.dram_tensor("x", [B_PER_CORE, N_NODES, F], F32, kind="ExternalInput")
    w_d = nc.dram_tensor("W", [F, F], F32, kind="ExternalInput")
    a_d = nc.dram_tensor("a", [2 * F, 1], F32, kind="ExternalInput")
    beta_d = nc.dram_tensor("beta", [1], F32, kind="ExternalInput")
    out_d = nc.dram_tensor("out", [B_PER_CORE, N_NODES, F], F32, kind="ExternalOutput")
    # scratch for row->per-partition-column roundtrips
    r_d = nc.dram_tensor("r_scratch", [B_PER_CORE, N_NODES], F32)
    s_d = nc.dram_tensor("s_scratch", [B_PER_CORE, N_NODES], F32)

    with TileContext(nc) as tc, ExitStack() as ctx:
        # ---------------- pools ----------------
        singles = ctx.enter_context(tc.tile_pool(name="singles", bufs=1))
        xin = ctx.enter_context(tc.tile_pool(name="xin", bufs=16))
        xtp = ctx.enter_context(tc.tile_pool(name="xtp", bufs=2))  # xT bf16
        hpool = ctx.enter_context(tc.tile_pool(name="hpool", bufs=16))
        spool = ctx.enter_context(tc.tile_pool(name="spool", bufs=2))
        utp = ctx.enter_context(tc.tile_pool(name="utp", bufs=16))
        tpool = ctx.enter_context(tc.tile_pool(name="tpool", bufs=3))
        qp = ctx.enter_context(tc.tile_pool(name="qp", bufs=2))
        epool = ctx.enter_context(tc.tile_pool(name="epool", bufs=4))
        # PSUM: ps_tr 2x[128,512](2 banks) ps_mm 3x[128,512](3) ps_s [2,1024](2)
        # ps_e 1x[128,512](1) -> 8 banks
        ps_tr = ctx.enter_context(tc.tile_pool(name="ps_tr", bufs=2, space="PSUM"))
        ps_mm = ctx.enter_context(tc.tile_pool(name="ps_mm", bufs=3, space="PSUM"))
        ps_s = ctx.enter_context(tc.tile_pool(name="ps_s", bufs=1, space="PSUM"))
        ps_e = ctx.enter_context(tc.tile_pool(name="ps_e", bufs=1, space="PSUM"))

        # ---------------- prologue ----------------
        identf = singles.tile([P, P], F32, tag="identf")
        make_identity(nc, identf)
        ident = singles.tile([P, P], F32, tag="ident")
        nc.scalar.copy(out=_r(ident), in_=identf)

        ones2b = singles.tile([P, 2], BF16, tag="ones2b")
        nc.gpsimd.memset(ones2b, 1.0)
        # [1,128] ones row, f32r-written: stationary for the K=1 E-broadcast
        onesrowf = singles.tile([1, P], F32, tag="onesrowf")
        nc.gpsimd.memset(onesrowf, 1.0)
        onesrow = singles.tile([1, P], F32, tag="onesrow")
        nc.scalar.copy(out=_r(onesrow), in_=onesrowf)

        a_flat = a_d.rearrange("f one -> (f one)")
        a1b = singles.tile([P, F], F32, tag="a1b")
        a2b = singles.tile([P, F], F32, tag="a2b")
        beta_sb = singles.tile([1, 1], F32, tag="beta_sb")
        w_sb = []
        wb = []
        for k in range(NK):
            wk = singles.tile([P, F], F32, tag=f"w_sb{k}")
            w_sb.append(wk)
            wbk = singles.tile([P, F], BF16, tag=f"wb{k}")
            wb.append(wbk)
        w12b = singles.tile([P, 2 * NK], BF16, tag="w12b")

        def load_weights():
            nc.sync.dma_start(out=a1b, in_=a_flat[0:F].partition_broadcast(P))
            nc.sync.dma_start(out=a2b, in_=a_flat[F : 2 * F].partition_broadcast(P))
            # beta lands in SBUF only to keep the input bound (value baked)
            nc.sync.dma_start(out=beta_sb, in_=beta_d[0:1].unsqueeze(0))
            for k in range(NK):
                nc.sync.dma_start(out=w_sb[k], in_=w_d[k * P : (k + 1) * P, :])
                # bf16 copy of W for the h matmul (moving operand)
                nc.scalar.copy(out=wb[k], in_=w_sb[k])
                w12f = qp.tile([P, 2], F32, tag="w12f")
                prod = qp.tile([P, F], F32, tag="wa_prod")
                for j, ab in enumerate((a1b, a2b)):
                    nc.vector.tensor_tensor(
                        out=prod, in0=w_sb[k], in1=ab, op=AL.mult
                    )
                    nc.vector.reduce_sum(
                        out=w12f[:, j : j + 1], in_=prod, axis=mybir.AxisListType.X
                    )
                nc.scalar.copy(out=w12b[:, 2 * k : 2 * k + 2], in_=w12f)

        # ---------------- PE warm-up ----------------
        # hold the activity monitor busy during the initial DMA window so
        # real matmuls start at the max clock
        for _ in range(4):
            wp = ps_tr.tile([P, F], F32, tag="ps_tr")
            nc.tensor.transpose(_r(wp[:, 0:P]), _r(ident), _r(ident))
            nc.tensor.transpose(_r(wp[:, P : 2 * P]), _r(ident), _r(ident))

        # ---------------- per-batch state ----------------
        xt_alls = {}
        h_sbs = {}
        uts = {}
        rcols = {}
        e1bs = {}
        e1abs = {}
        e2cols = {}
        e2acols = {}
        x_tiles = {}

        def phase_A_dma(b):  # issue all x loads for this batch
            x_tiles[b] = []
            for n in range(NN):
                x_t = xin.tile([P, F], F32, tag="x_t")
                nc.sync.dma_start(out=_r(x_t), in_=_r(x_d[b, n * P : (n + 1) * P, :]))
                x_tiles[b].append(x_t)

        def phase_T(b):  # transpose x into bf16 xT
            xt_all = xtp.tile([P, NK * N_NODES], BF16, tag="xt_all")
            xt_alls[b] = xt_all
            for n in range(NN):
                x_t = x_tiles[b][n]
                xp = ps_tr.tile([P, F], F32, tag="ps_tr")
                for k in range(NK):
                    nc.tensor.transpose(
                        _r(xp[:, k * P : (k + 1) * P]),
                        _r(x_t[:, k * P : (k + 1) * P]),
                        _r(ident),
                    )
                dst = xt_all.rearrange("p (k c) -> p k c", k=NK)[
                    :, :, n * P : (n + 1) * P
                ]
                src = xp.rearrange("p (k c) -> p k c", k=NK)
                # batch 0: DVE is idle this early; batch 1: DVE is busy
                # with C0, ACT is free after the h0 copies
                if b == 0:
                    nc.vector.tensor_copy(out=dst, in_=src)
                else:
                    nc.scalar.copy(out=dst, in_=src)

        e1rows = {}
        e1arows = {}

        def phase_S(b):
            # s rows; u's factorization u = max(e^{s1}e^{s2},
            # e^{.2 s1}e^{.2 s2}) needs exps only on the s VECTORS, not on
            # the NxN matrix: E2/E2a as [128,8] columns (bias scalars),
            # E1/E1a as [1,N] rows that a K=1 PE matmul broadcasts to
            # [128,N] (a partition_broadcast DMA measured 12.5us; the PE
            # outer product is ~0.5us).
            xt_all = xt_alls[b]
            s_ps = ps_s.tile([2, N_NODES], F32, tag="ps_s")
            for k in range(NK):
                for hh in range(2):
                    nc.tensor.matmul(
                        s_ps[:, hh * F : (hh + 1) * F],
                        lhsT=w12b[:, 2 * k : 2 * k + 2],
                        rhs=xt_all[:, k * N_NODES + hh * F : k * N_NODES + (hh + 1) * F],
                        start=(k == 0),
                        stop=(k == NK - 1),
                    )
            s_sb = spool.tile([2, N_NODES], F32, tag="s_sb")
            nc.vector.tensor_copy(out=s_sb, in_=s_ps)
            # E1/E1a rows (f32r-written: they feed the broadcast matmul)
            e1row = spool.tile([1, N_NODES], F32, tag="e1row")
            e1rows[b] = e1row
            nc.scalar.activation(out=_r(e1row), in_=s_sb[0:1, :], func=AF.Exp)
            e1arow = spool.tile([1, N_NODES], F32, tag="e1arow")
            e1arows[b] = e1arow
            nc.scalar.activation(
                out=_r(e1arow), in_=s_sb[0:1, :], func=AF.Exp, scale=ALPHA
            )
            # s2 row -> per-partition columns through DRAM, then tiny exps
            nc.sync.dma_start(out=s_d[b].unsqueeze(0), in_=s_sb[1:2, :])
            s2col = spool.tile([P, NN], F32, tag="s2col")
            nc.sync.dma_start(out=s2col, in_=s_d[b].rearrange("(n p) -> p n", p=P))
            e2col = spool.tile([P, NN], F32, tag="e2col")
            nc.scalar.activation(out=e2col, in_=s2col, func=AF.Exp)
            e2cols[b] = e2col
            e2acol = spool.tile([P, NN], F32, tag="e2acol")
            nc.scalar.activation(out=e2acol, in_=s2col, func=AF.Exp, scale=ALPHA)
            e2acols[b] = e2acol

        def emit_E_bcast(b):
            # e1bf/e1abf[p, i] = E1/E1a[i] via ones-column outer product
            e1bf = spool.tile([P, N_NODES], F32, tag="e1bf")
            e1bs[b] = e1bf
            e1abf = spool.tile([P, N_NODES], F32, tag="e1abf")
            e1abs[b] = e1abf
            for row, dstf in ((e1rows[b], e1bf), (e1arows[b], e1abf)):
                for hh in range(2):
                    bp = ps_e.tile([P, F], F32, tag="ps_e")
                    nc.tensor.matmul(
                        bp,
                        lhsT=_r(onesrow),
                        rhs=_r(row[0:1, hh * F : (hh + 1) * F]),
                        start=True,
                        stop=True,
                    )
                    nc.vector.tensor_copy(
                        out=dstf[:, hh * F : (hh + 1) * F], in_=bp
                    )

        def emit_C_tile(b, j):
            # uT[j][p, i] = max(E1[i]E2[jp], E1a[i]E2a[jp]) -- 3 DVE ops in
            # 2x fast mode (SBUF-only operands; bf16 outs make the max
            # 2x_1p-eligible; per-partition scalars are dtype-exempt).
            # Zero ACT, zero PE.
            t1 = tpool.tile([P, N_NODES], BF16, tag="t1")
            nc.vector.tensor_scalar(
                out=t1, in0=e1bs[b], scalar1=e2cols[b][:, j : j + 1], scalar2=None,
                op0=AL.mult,
            )
            t2 = tpool.tile([P, N_NODES], BF16, tag="t2")
            nc.vector.tensor_scalar(
                out=t2, in0=e1abs[b], scalar1=e2acols[b][:, j : j + 1], scalar2=None,
                op0=AL.mult,
            )
            u = utp.tile([P, N_NODES], BF16, tag="ut")
            nc.vector.tensor_tensor(out=u, in0=t1, in1=t2, op=AL.max)
            uts[b][j] = u

        def phase_C(b):
            uts[b] = [None] * NN
            for j in range(NN):
                emit_C_tile(b, j)

        def phase_B(b):  # h = x @ W  (bf16 out for the p matmul + epilogue)
            xt_all = xt_alls[b]
            h_sbs[b] = []
            for n in range(NN):
                h_ps = ps_mm.tile([P, F], F32, tag="ps_mm")
                for k in range(NK):
                    nc.tensor.matmul(
                        h_ps,
                        lhsT=xt_all[:, k * N_NODES + n * P : k * N_NODES + (n + 1) * P],
                        rhs=wb[k],
                        start=(k == 0),
                        stop=(k == NK - 1),
                    )
                ht = hpool.tile([P, F], BF16, tag="h_sb")
                nc.scalar.copy(out=ht, in_=h_ps)
                h_sbs[b].append(ht)
                if n == 2:
                    # E-broadcast matmuls slot in mid-B so the E rows (ACT)
                    # are ready and the C phase can start ~6 tiles early
                    emit_E_bcast(b)

        def phase_R(b):  # rowsum -> reciprocal columns
            ut = uts[b]
            rs_ps = ps_s.tile([2, N_NODES], F32, tag="ps_s")
            for j in range(NN):
                for hh in range(2):
                    nc.tensor.matmul(
                        rs_ps[:, hh * F : (hh + 1) * F],
                        lhsT=ones2b,
                        rhs=ut[j][:, hh * F : (hh + 1) * F],
                        start=(j == 0),
                        stop=(j == NN - 1),
                    )
            rrow = spool.tile([1, N_NODES], F32, tag="rrow")
            nc.vector.tensor_copy(out=rrow, in_=rs_ps[0:1, :])
            nc.sync.dma_start(out=r_d[b].unsqueeze(0), in_=rrow)
            rcraw = spool.tile([P, NN], F32, tag="rcraw")
            nc.sync.dma_start(out=rcraw, in_=r_d[b].rearrange("(n p) -> p n", p=P))
            rcol = spool.tile([P, NN], F32, tag="rcol")
            rcols[b] = rcol
            nc.vector.reciprocal(out=rcol, in_=rcraw)

        def emit_DE_tile(b, n):  # p[n] = u @ h + fused ELU epilogue
            ut, h_sb, rcol = uts[b], h_sbs[b], rcols[b]
            p_ps = ps_mm.tile([P, F], F32, tag="ps_mm")
            for j in range(NN):
                nc.tensor.matmul(
                    p_ps,
                    lhsT=ut[j][:, n * P : (n + 1) * P],
                    rhs=h_sb[j],
                    start=(j == 0),
                    stop=(j == NN - 1),
                )
            hin = h_sb[n]
            if beta_val != 1.0:
                hb = epool.tile([P, F], F32, tag="hb")
                nc.vector.tensor_scalar_mul(hb, hin, float(beta_val))
                hin = hb
            v = epool.tile([P, F], BF16, tag="v")
            # v = p * (1/rowsum) + beta*h (the one unavoidable 1x stt)
            nc.vector.scalar_tensor_tensor(
                out=v, in0=p_ps, scalar=rcol[:, n : n + 1], in1=hin,
                op0=AL.mult, op1=AL.add,
            )
            # elu(v) = max(exp(min(v,0)) - 1, v); min(v,0) = -relu(-v) on ACT
            rl = epool.tile([P, F], BF16, tag="rl")
            nc.scalar.activation(out=rl, in_=v, func=AF.Relu, scale=-1.0)
            em = epool.tile([P, F], BF16, tag="em")
            nc.scalar.activation(out=em, in_=rl, func=AF.Exp, scale=-1.0)
            o1 = epool.tile([P, F], BF16, tag="o1")
            nc.vector.tensor_scalar(
                out=o1, in0=em, scalar1=-1.0, scalar2=None, op0=AL.add
            )
            o = epool.tile([P, F], F32, tag="o")
            nc.vector.tensor_tensor(out=o, in0=o1, in1=v, op=AL.max)
            nc.sync.dma_start(out=out_d[b, n * P : (n + 1) * P, :], in_=o)

        # ------------- software-pipelined emission -------------
        # PE order: warmup T0 S0 B0 T1 S1 B1 R0 DE0 R1 DE1 -- back-to-back
        # matmuls, never paced by ACT/DVE.  C phases are pure ACT/DVE and
        # run concurrently (C0 under B0/T1, C1 under B1/DE0).
        phase_A_dma(0)
        load_weights()
        phase_T(0)
        phase_S(0)
        phase_A_dma(1)
        phase_B(0)
        phase_T(1)
        phase_C(0)
        phase_S(1)
        phase_B(1)
        phase_R(0)
        # interleave C1 (DVE) with DE0 emission so neither epi0 nor C1
        # monopolizes the DVE queue; C1 is 2:1 front-loaded so it finishes
        # before R1 needs the u1 tiles
        uts[1] = [None] * NN
        c1_next = 0
        for n in range(NN):
            while c1_next < min(NN, 2 * (n + 1)):
                emit_C_tile(1, c1_next)
                c1_next += 1
            if n == 5:
                phase_R(1)
            emit_DE_tile(0, n)
        for n in range(NN):
            emit_DE_tile(1, n)

    nc.finalize()
    return nc


_NC_CACHE = {}


def _get_nc(beta_val: float) -> bass.Bass:
    key = float(beta_val)
    if key not in _NC_CACHE:
        _NC_CACHE[key] = build_nc(beta_val=key)
    return _NC_CACHE[key]


def kernel(x, W, a, beta, _trace=False, _mm_fp32=False):
    x = np.ascontiguousarray(x, dtype=np.float32)
    W = np.ascontiguousarray(W, dtype=np.float32)
    a = np.ascontiguousarray(a, dtype=np.float32)
    beta = np.ascontiguousarray(beta, dtype=np.float32)

    nc = _get_nc(float(beta.reshape(-1)[0]))
    in_maps = [
        {
            "x": x[c * B_PER_CORE : (c + 1) * B_PER_CORE],
            "W": W,
            "a": a,
            "beta": beta,
        }
        for c in range(N_CORES)
    ]
    res = run_bass_kernel_spmd(nc, in_maps, core_ids=list(range(N_CORES)), trace=_trace)
    out = np.concatenate(
        [np.asarray(r["out"]).astype(np.float32) for r in res.results], axis=0
    )
    if _trace:
        kernel.last_exec_time_ns = res.exec_time_ns
        kernel.last_results = res
    return out


if __name__ == "__main__":
    rng = np.random.default_rng(0)
    x = rng.standard_normal((B_TOTAL, N_NODES, F), dtype=np.float32)
    W = rng.standard_normal((F, F), dtype=np.float32) * 0.05
    a = rng.standard_normal((2 * F, 1), dtype=np.float32) * 0.05
    beta = np.ones((1,), dtype=np.float32)
    out = kernel(x, W, a, beta)
    print("out", out.shape, out.dtype)


# revision 35
# speedup vs baseline: 1.3080x; 1.0532x over previous
"""Trainium2 Bass kernel for a batched GAT layer (BGATLayer).

Reference computation (per batch b of B=16, N=1024 nodes, F=512 features):
    h   = x @ W                                   # [N, F]
    s1  = h @ a1 ; s2 = h @ a2                    # [N]
    e   = leakyrelu(s1[:,None] + s2[None,:], 0.2) # [N, N]
    att = softmax(e, axis=1)                      # row softmax
    out = elu(att @ h + beta * h)                 # [N, F]

Sharding: batch B=16 split across 8 NeuronCores (2 batches/core, data
parallel); W/a/beta replicated.

v2 design (v1 measured 147us; PE busy 107us incl ~40us at HAM half
clock from C-phase starvation, plus a 19us epilogue tail):
  * All matmul operands are bf16 (tolerance is 2e-2; bf16 adds ~4e-3).
    Streaming rate is the same 1 cyc/row as f32r@512, but LDWEIGHTS is
    ~2x faster and SBUF footprint halves.
  * uT tiles are computed with ZERO PE work: uT[j][p,i] =
    exp(prelu(s1[i] + s2[j*128+p])).  s1 enters as a [128, N]
    partition-broadcast tile (SBUF->SBUF DMA), s2 as a per-partition
    bias column (ACT activation bias= accepts a [128,1] AP; DVE path
    uses tensor_scalar with an AP scalar).  v1 computed each z tile as
    a K=2 PE matmul that ping-ponged with ACT and starved the PE into
    the HAM's k=4/8 duty-cycle downclock.
  * PE stream is matmul-only and back-to-back:
      warmup T0 S0 B0 T1 S1 B1 R0 DE0(p) R1 DE1(p)
    so the activity monitor keeps the clock at max.
  * rowsum still via ones-stationary matmuls (cheap: 2-row stationary),
    reciprocal through the DRAM row->column roundtrip.
  * epilogue per tile: v = p*recip + h (DVE stt, mixed f32/bf16),
    m = min(v,0) (DVE), em = exp(m) (ACT), out = max(em-1, v) (DVE),
    trailing the p matmuls tile-by-tile instead of bunching at the end.
"""

import sys

sys.path.insert(0, "/opt/trn_rl_repo")

from contextlib import ExitStack

import numpy as np

import concourse.bacc as bacc
import concourse.bass as bass
import concourse.mybir as mybir
from concourse.bass_utils import run_bass_kernel_spmd
from concourse.masks import make_identity
from concourse.tile import TileContext

P = 128
N_NODES = 1024
F = 512
B_TOTAL = 16
N_CORES = 8
B_PER_CORE = B_TOTAL // N_CORES
NK = F // P  # 4 contraction chunks for x @ W
NN = N_NODES // P  # 8 node chunks
ALPHA = 0.2

F32 = mybir.dt.float32
F32R = mybir.dt.float32r
BF16 = mybir.dt.bfloat16
AL = mybir.AluOpType
AF = mybir.ActivationFunctionType


def _r(ap):
    """float32r view of an fp32 AP (PE reduced-precision matmul mode)."""
    return ap.bitcast(F32R)


def build_nc(beta_val: float = 1.0) -> bass.Bass:
    nc = bacc.Bacc("TRN2")
    x_d = nc.dram_tensor("x", [B_PER_CORE, N_NODES, F], F32, kind="ExternalInput")
    w_d = nc.dram_tensor("W", [F, F], F32, kind="ExternalInput")
    a_d = nc.dram_tensor("a", [2 * F, 1], F32, kind="ExternalInput")
    beta_d = nc.dram_tensor("beta", [1], F32, kind="ExternalInput")
    out_d = nc.dram_tensor("out", [B_PER_CORE, N_NODES, F], F32, kind="ExternalOutput")
    # scratch for row->per-partition-column roundtrips
    r_d = nc.dram_tensor("r_scratch", [B_PER_CORE, N_NODES], F32)
    s_d = nc.dram_tensor("s_scratch", [B_PER_CORE, N_NODES], F32)

    with TileContext(nc) as tc, ExitStack() as ctx:
        # ---------------- pools ----------------
        singles = ctx.enter_context(tc.tile_pool(name="singles", bufs=1))
        xin = ctx.enter_context(tc.tile_pool(name="xin", bufs=16))
        xtp = ctx.enter_context(tc.tile_pool(name="xtp", bufs=2))  # xT bf16
        hpool = ctx.enter_context(tc.tile_pool(name="hpool", bufs=16))
        spool = ctx.enter_context(tc.tile_pool(name="spool", bufs=2))
        utp = ctx.enter_context(tc.tile_pool(name="utp", bufs=16))
        tpool = ctx.enter_context(tc.tile_pool(name="tpool", bufs=3))
        qp = ctx.enter_context(tc.tile_pool(name="qp", bufs=2))
        epool = ctx.enter_context(tc.tile_pool(name="epool", bufs=4))
        # PSUM: ps_tr 2x[128,512](2 banks) ps_mm 3x[128,512](3) ps_s [2,1024](2)
        # ps_e 1x[128,512](1) -> 8 banks
        ps_tr = ctx.enter_context(tc.tile_pool(name="ps_tr", bufs=2, space="PSUM"))
        ps_mm = ctx.enter_context(tc.tile_pool(name="ps_mm", bufs=3, space="PSUM"))
        ps_s = ctx.enter_context(tc.tile_pool(name="ps_s", bufs=1, space="PSUM"))
        ps_e = ctx.enter_context(tc.tile_pool(name="ps_e", bufs=1, space="PSUM"))

        # ---------------- prologue ----------------
        identf = singles.tile([P, P], F32, tag="identf")
        make_identity(nc, identf)
        ident = singles.tile([P, P], F32, tag="ident")
        nc.scalar.copy(out=_r(ident), in_=identf)

        ones2b = singles.tile([P, 2], BF16, tag="ones2b")
        nc.gpsimd.memset(ones2b, 1.0)
        # [1,128] ones row, f32r-written: stationary for the K=1 E-broadcast
        onesrowf = singles.tile([1, P], F32, tag="onesrowf")
        nc.gpsimd.memset(onesrowf, 1.0)
        onesrow = singles.tile([1, P], F32, tag="onesrow")
        nc.scalar.copy(out=_r(onesrow), in_=onesrowf)

        a_flat = a_d.rearrange("f one -> (f one)")
        a1b = singles.tile([P, F], F32, tag="a1b")
        a2b = singles.tile([P, F], F32, tag="a2b")
        beta_sb = singles.tile([1, 1], F32, tag="beta_sb")
        w_sb = []
        wb = []
        for k in range(NK):
            wk = singles.tile([P, F], F32, tag=f"w_sb{k}")
            w_sb.append(wk)
            wbk = singles.tile([P, F], BF16, tag=f"wb{k}")
            wb.append(wbk)
        w12b = singles.tile([P, 2 * NK], BF16, tag="w12b")

        def load_weights():
            nc.sync.dma_start(out=a1b, in_=a_flat[0:F].partition_broadcast(P))
            nc.sync.dma_start(out=a2b, in_=a_flat[F : 2 * F].partition_broadcast(P))
            # beta lands in SBUF only to keep the input bound (value baked)
            nc.sync.dma_start(out=beta_sb, in_=beta_d[0:1].unsqueeze(0))
            for k in range(NK):
                nc.sync.dma_start(out=w_sb[k], in_=w_d[k * P : (k + 1) * P, :])
                # bf16 copy of W for the h matmul (moving operand)
                nc.scalar.copy(out=wb[k], in_=w_sb[k])
                w12f = qp.tile([P, 2], F32, tag="w12f")
                prod = qp.tile([P, F], F32, tag="wa_prod")
                for j, ab in enumerate((a1b, a2b)):
                    nc.vector.tensor_tensor(
                        out=prod, in0=w_sb[k], in1=ab, op=AL.mult
                    )
                    nc.vector.reduce_sum(
                        out=w12f[:, j : j + 1], in_=prod, axis=mybir.AxisListType.X
                    )
                nc.scalar.copy(out=w12b[:, 2 * k : 2 * k + 2], in_=w12f)

        # ---------------- PE warm-up ----------------
        # hold the activity monitor busy during the initial DMA window so
        # real matmuls start at the max clock
        for _ in range(10):
            wp = ps_tr.tile([P, F], F32, tag="ps_tr")
            nc.tensor.transpose(_r(wp[:, 0:P]), _r(ident), _r(ident))
            nc.tensor.transpose(_r(wp[:, P : 2 * P]), _r(ident), _r(ident))

        # ---------------- per-batch state ----------------
        xt_alls = {}
        h_sbs = {}
        uts = {}
        rcols = {}
        e1bs = {}
        e1abs = {}
        e2cols = {}
        e2acols = {}
        x_tiles = {}

        def phase_A_dma(b):  # issue all x loads for this batch
            x_tiles[b] = []
            for n in range(NN):
                x_t = xin.tile([P, F], F32, tag="x_t")
                nc.sync.dma_start(out=_r(x_t), in_=_r(x_d[b, n * P : (n + 1) * P, :]))
                x_tiles[b].append(x_t)

        def phase_T(b):  # transpose x into bf16 xT
            xt_all = xtp.tile([P, NK * N_NODES], BF16, tag="xt_all")
            xt_alls[b] = xt_all
            for n in range(NN):
                x_t = x_tiles[b][n]
                xp = ps_tr.tile([P, F], F32, tag="ps_tr")
                for k in range(NK):
                    nc.tensor.transpose(
                        _r(xp[:, k * P : (k + 1) * P]),
                        _r(x_t[:, k * P : (k + 1) * P]),
                        _r(ident),
                    )
                dst = xt_all.rearrange("p (k c) -> p k c", k=NK)[
                    :, :, n * P : (n + 1) * P
                ]
                src = xp.rearrange("p (k c) -> p k c", k=NK)
                # ACT: DVE is the scarce engine (C phase + epilogues)
                nc.scalar.copy(out=dst, in_=src)

        e1rows = {}
        e1arows = {}

        def phase_S(b):
            # s rows; u's factorization u = max(e^{s1}e^{s2},
            # e^{.2 s1}e^{.2 s2}) needs exps only on the s VECTORS, not on
            # the NxN matrix: E2/E2a as [128,8] columns (bias scalars),
            # E1/E1a as [1,N] rows that a K=1 PE matmul broadcasts to
            # [128,N] (a partition_broadcast DMA measured 12.5us; the PE
            # outer product is ~0.5us).
            xt_all = xt_alls[b]
            s_ps = ps_s.tile([2, N_NODES], F32, tag="ps_s")
            for k in range(NK):
                for hh in range(2):
                    nc.tensor.matmul(
                        s_ps[:, hh * F : (hh + 1) * F],
                        lhsT=w12b[:, 2 * k : 2 * k + 2],
                        rhs=xt_all[:, k * N_NODES + hh * F : k * N_NODES + (hh + 1) * F],
                        start=(k == 0),
                        stop=(k == NK - 1),
                    )
            s_sb = spool.tile([2, N_NODES], F32, tag="s_sb")
            nc.vector.tensor_copy(out=s_sb, in_=s_ps)
            # E1/E1a rows (f32r-written: they feed the broadcast matmul)
            e1row = spool.tile([1, N_NODES], F32, tag="e1row")
            e1rows[b] = e1row
            nc.scalar.activation(out=_r(e1row), in_=s_sb[0:1, :], func=AF.Exp)
            e1arow = spool.tile([1, N_NODES], F32, tag="e1arow")
            e1arows[b] = e1arow
            nc.scalar.activation(
                out=_r(e1arow), in_=s_sb[0:1, :], func=AF.Exp, scale=ALPHA
            )
            # s2 row -> per-partition columns through DRAM, then tiny exps
            nc.sync.dma_start(out=s_d[b].unsqueeze(0), in_=s_sb[1:2, :])
            s2col = spool.tile([P, NN], F32, tag="s2col")
            nc.sync.dma_start(out=s2col, in_=s_d[b].rearrange("(n p) -> p n", p=P))
            e2col = spool.tile([P, NN], F32, tag="e2col")
            nc.scalar.activation(out=e2col, in_=s2col, func=AF.Exp)
            e2cols[b] = e2col
            e2acol = spool.tile([P, NN], F32, tag="e2acol")
            nc.scalar.activation(out=e2acol, in_=s2col, func=AF.Exp, scale=ALPHA)
            e2acols[b] = e2acol

        def emit_E_bcast(b):
            # e1b/e1ab[p, i] = E1/E1a[i] via ones-column outer product;
            # bf16 SBUF copies so the C-phase tensor_scalars hit 4x_2p
            e1b = spool.tile([P, N_NODES], BF16, tag="e1b")
            e1bs[b] = e1b
            e1ab = spool.tile([P, N_NODES], BF16, tag="e1ab")
            e1abs[b] = e1ab
            for row, dstf in ((e1rows[b], e1b), (e1arows[b], e1ab)):
                for hh in range(2):
                    bp = ps_e.tile([P, F], F32, tag="ps_e")
                    nc.tensor.matmul(
                        bp,
                        lhsT=_r(onesrow),
                        rhs=_r(row[0:1, hh * F : (hh + 1) * F]),
                        start=True,
                        stop=True,
                    )
                    nc.scalar.copy(out=dstf[:, hh * F : (hh + 1) * F], in_=bp)

        def emit_C_tile(b, j):
            # uT[j][p, i] = max(E1[i]E2[jp], E1a[i]E2a[jp]) -- 3 DVE ops in
            # 2x fast mode (SBUF-only operands; bf16 outs make the max
            # 2x_1p-eligible; per-partition scalars are dtype-exempt).
            # Zero ACT, zero PE.
            t1 = tpool.tile([P, N_NODES], BF16, tag="t1")
            nc.vector.tensor_scalar(
                out=t1, in0=e1bs[b], scalar1=e2cols[b][:, j : j + 1], scalar2=None,
                op0=AL.mult,
            )
            t2 = tpool.tile([P, N_NODES], BF16, tag="t2")
            nc.vector.tensor_scalar(
                out=t2, in0=e1abs[b], scalar1=e2acols[b][:, j : j + 1], scalar2=None,
                op0=AL.mult,
            )
            u = utp.tile([P, N_NODES], BF16, tag="ut")
            nc.vector.tensor_tensor(out=u, in0=t1, in1=t2, op=AL.max)
            uts[b][j] = u

        def phase_C(b):
            uts[b] = [None] * NN
            for j in range(NN):
                emit_C_tile(b, j)

        def phase_B(b):  # h = x @ W  (bf16 out for the p matmul + epilogue)
            xt_all = xt_alls[b]
            h_sbs[b] = []
            for n in range(NN):
                h_ps = ps_mm.tile([P, F], F32, tag="ps_mm")
                for k in range(NK):
                    nc.tensor.matmul(
                        h_ps,
                        lhsT=xt_all[:, k * N_NODES + n * P : k * N_NODES + (n + 1) * P],
                        rhs=wb[k],
                        start=(k == 0),
                        stop=(k == NK - 1),
                    )
                ht = hpool.tile([P, F], BF16, tag="h_sb")
                nc.scalar.copy(out=ht, in_=h_ps)
                h_sbs[b].append(ht)
                if n == 2:
                    # E-broadcast matmuls slot in mid-B so the E rows (ACT)
                    # are ready and the C phase can start ~6 tiles early
                    emit_E_bcast(b)

        def phase_R(b):  # rowsum -> reciprocal columns
            ut = uts[b]
            rs_ps = ps_s.tile([2, N_NODES], F32, tag="ps_s")
            for j in range(NN):
                for hh in range(2):
                    nc.tensor.matmul(
                        rs_ps[:, hh * F : (hh + 1) * F],
                        lhsT=ones2b,
                        rhs=ut[j][:, hh * F : (hh + 1) * F],
                        start=(j == 0),
                        stop=(j == NN - 1),
                    )
            rrow = spool.tile([1, N_NODES], F32, tag="rrow")
            nc.vector.tensor_copy(out=rrow, in_=rs_ps[0:1, :])
            nc.sync.dma_start(out=r_d[b].unsqueeze(0), in_=rrow)
            rcraw = spool.tile([P, NN], F32, tag="rcraw")
            nc.sync.dma_start(out=rcraw, in_=r_d[b].rearrange("(n p) -> p n", p=P))
            rcol = spool.tile([P, NN], F32, tag="rcol")
            rcols[b] = rcol
            nc.vector.reciprocal(out=rcol, in_=rcraw)

        def emit_DE_tile(b, n):  # p[n] = u @ h + fused ELU epilogue
            ut, h_sb, rcol = uts[b], h_sbs[b], rcols[b]
            p_ps = ps_mm.tile([P, F], F32, tag="ps_mm")
            for j in range(NN):
                nc.tensor.matmul(
                    p_ps,
                    lhsT=ut[j][:, n * P : (n + 1) * P],
                    rhs=h_sb[j],
                    start=(j == 0),
                    stop=(j == NN - 1),
                )
            hin = h_sb[n]
            if beta_val != 1.0:
                hb = epool.tile([P, F], BF16, tag="hb")
                nc.vector.tensor_scalar_mul(hb, hin, float(beta_val))
                hin = hb
            # v = p*(1/rowsum) + beta*h, split ACT scale-copy + DVE 2x add
            v1 = epool.tile([P, F], BF16, tag="v1")
            nc.scalar.activation(
                out=v1, in_=p_ps, func=AF.Copy, scale=rcol[:, n : n + 1]
            )
            v = epool.tile([P, F], BF16, tag="v")
            nc.vector.tensor_tensor(out=v, in0=v1, in1=hin, op=AL.add)
            # elu(v) = max(exp(min(v,0)) - 1, v); min(v,0) = -relu(-v) on ACT
            rl = epool.tile([P, F], BF16, tag="rl")
            nc.scalar.activation(out=rl, in_=v, func=AF.Relu, scale=-1.0)
            em = epool.tile([P, F], BF16, tag="em")
            nc.scalar.activation(out=em, in_=rl, func=AF.Exp, scale=-1.0)
            o1 = epool.tile([P, F], BF16, tag="o1")
            nc.vector.tensor_scalar(
                out=o1, in0=em, scalar1=-1.0, scalar2=None, op0=AL.add
            )
            o = epool.tile([P, F], F32, tag="o")
            nc.vector.tensor_tensor(out=o, in0=o1, in1=v, op=AL.max)
            nc.sync.dma_start(out=out_d[b, n * P : (n + 1) * P, :], in_=o)

        # ------------- software-pipelined emission -------------
        # PE order: warmup T0 S0 B0 T1 S1 B1 R0 DE0 R1 DE1 -- back-to-back
        # matmuls, never paced by ACT/DVE.  C phases are pure ACT/DVE and
        # run concurrently (C0 under B0/T1, C1 under B1/DE0).
        phase_A_dma(0)
        load_weights()
        phase_T(0)
        phase_S(0)
        phase_A_dma(1)
        phase_B(0)
        phase_T(1)
        phase_C(0)
        phase_S(1)
        phase_B(1)
        phase_R(0)
        # interleave C1 (DVE) with DE0 emission so neither epi0 nor C1
        # monopolizes the DVE queue; C1 is 2:1 front-loaded so it finishes
        # before R1 needs the u1 tiles
        uts[1] = [None] * NN
        c1_next = 0
        for n in range(NN):
            while c1_next < min(NN, 2 * (n + 1)):
                emit_C_tile(1, c1_next)
                c1_next += 1
            if n == 5:
                phase_R(1)
            emit_DE_tile(0, n)
        for n in range(NN):
            emit_DE_tile(1, n)

    nc.finalize()
    return nc


_NC_CACHE = {}


def _get_nc(beta_val: float) -> bass.Bass:
    key = float(beta_val)
    if key not in _NC_CACHE:
        _NC_CACHE[key] = build_nc(beta_val=key)
    return _NC_CACHE[key]


def kernel(x, W, a, beta, _trace=False, _mm_fp32=False):
    x = np.ascontiguousarray(x, dtype=np.float32)
    W = np.ascontiguousarray(W, dtype=np.float32)
    a = np.ascontiguousarray(a, dtype=np.float32)
    beta = np.ascontiguousarray(beta, dtype=np.float32)

    nc = _get_nc(float(beta.reshape(-1)[0]))
    in_maps = [
        {
            "x": x[c * B_PER_CORE : (c + 1) * B_PER_CORE],
            "W": W,
            "a": a,
            "beta": beta,
        }
        for c in range(N_CORES)
    ]
    res = run_bass_kernel_spmd(nc, in_maps, core_ids=list(range(N_CORES)), trace=_trace)
    out = np.concatenate(
        [np.asarray(r["out"]).astype(np.float32) for r in res.results], axis=0
    )
    if _trace:
        kernel.last_exec_time_ns = res.exec_time_ns
        kernel.last_results = res
    return out


if __name__ == "__main__":
    rng = np.random.default_rng(0)
    x = rng.standard_normal((B_TOTAL, N_NODES, F), dtype=np.float32)
    W = rng.standard_normal((F, F), dtype=np.float32) * 0.05
    a = rng.standard_normal((2 * F, 1), dtype=np.float32) * 0.05
    beta = np.ones((1,), dtype=np.float32)
    out = kernel(x, W, a, beta)
    print("out", out.shape, out.dtype)


# revision 37
# speedup vs baseline: 1.3155x; 1.0058x over previous
"""Trainium2 Bass kernel for a batched GAT layer (BGATLayer).

Reference computation (per batch b of B=16, N=1024 nodes, F=512 features):
    h   = x @ W                                   # [N, F]
    s1  = h @ a1 ; s2 = h @ a2                    # [N]
    e   = leakyrelu(s1[:,None] + s2[None,:], 0.2) # [N, N]
    att = softmax(e, axis=1)                      # row softmax
    out = elu(att @ h + beta * h)                 # [N, F]

Sharding: batch B=16 split across 8 NeuronCores (2 batches/core, data
parallel); W/a/beta replicated.

v2 design (v1 measured 147us; PE busy 107us incl ~40us at HAM half
clock from C-phase starvation, plus a 19us epilogue tail):
  * All matmul operands are bf16 (tolerance is 2e-2; bf16 adds ~4e-3).
    Streaming rate is the same 1 cyc/row as f32r@512, but LDWEIGHTS is
    ~2x faster and SBUF footprint halves.
  * uT tiles are computed with ZERO PE work: uT[j][p,i] =
    exp(prelu(s1[i] + s2[j*128+p])).  s1 enters as a [128, N]
    partition-broadcast tile (SBUF->SBUF DMA), s2 as a per-partition
    bias column (ACT activation bias= accepts a [128,1] AP; DVE path
    uses tensor_scalar with an AP scalar).  v1 computed each z tile as
    a K=2 PE matmul that ping-ponged with ACT and starved the PE into
    the HAM's k=4/8 duty-cycle downclock.
  * PE stream is matmul-only and back-to-back:
      warmup T0 S0 B0 T1 S1 B1 R0 DE0(p) R1 DE1(p)
    so the activity monitor keeps the clock at max.
  * rowsum still via ones-stationary matmuls (cheap: 2-row stationary),
    reciprocal through the DRAM row->column roundtrip.
  * epilogue per tile: v = p*recip + h (DVE stt, mixed f32/bf16),
    m = min(v,0) (DVE), em = exp(m) (ACT), out = max(em-1, v) (DVE),
    trailing the p matmuls tile-by-tile instead of bunching at the end.
"""

import sys

sys.path.insert(0, "/opt/trn_rl_repo")

from contextlib import ExitStack

import numpy as np

import concourse.bacc as bacc
import concourse.bass as bass
import concourse.mybir as mybir
from concourse.bass_utils import run_bass_kernel_spmd
from concourse.masks import make_identity
from concourse.tile import TileContext

P = 128
N_NODES = 1024
F = 512
B_TOTAL = 16
N_CORES = 8
B_PER_CORE = B_TOTAL // N_CORES
NK = F // P  # 4 contraction chunks for x @ W
NN = N_NODES // P  # 8 node chunks
ALPHA = 0.2

F32 = mybir.dt.float32
F32R = mybir.dt.float32r
BF16 = mybir.dt.bfloat16
AL = mybir.AluOpType
AF = mybir.ActivationFunctionType


def _r(ap):
    """float32r view of an fp32 AP (PE reduced-precision matmul mode)."""
    return ap.bitcast(F32R)


def build_nc(beta_val: float = 1.0) -> bass.Bass:
    nc = bacc.Bacc("TRN2")
    x_d = nc.dram_tensor("x", [B_PER_CORE, N_NODES, F], F32, kind="ExternalInput")
    w_d = nc.dram_tensor("W", [F, F], F32, kind="ExternalInput")
    a_d = nc.dram_tensor("a", [2 * F, 1], F32, kind="ExternalInput")
    beta_d = nc.dram_tensor("beta", [1], F32, kind="ExternalInput")
    out_d = nc.dram_tensor("out", [B_PER_CORE, N_NODES, F], F32, kind="ExternalOutput")
    # scratch for row->per-partition-column roundtrips
    r_d = nc.dram_tensor("r_scratch", [B_PER_CORE, N_NODES], F32)
    s_d = nc.dram_tensor("s_scratch", [B_PER_CORE, N_NODES], F32)

    with TileContext(nc) as tc, ExitStack() as ctx:
        # ---------------- pools ----------------
        singles = ctx.enter_context(tc.tile_pool(name="singles", bufs=1))
        xin = ctx.enter_context(tc.tile_pool(name="xin", bufs=16))
        xtp = ctx.enter_context(tc.tile_pool(name="xtp", bufs=2))  # xT bf16
        hpool = ctx.enter_context(tc.tile_pool(name="hpool", bufs=16))
        spool = ctx.enter_context(tc.tile_pool(name="spool", bufs=2))
        utp = ctx.enter_context(tc.tile_pool(name="utp", bufs=16))
        tpool = ctx.enter_context(tc.tile_pool(name="tpool", bufs=3))
        qp = ctx.enter_context(tc.tile_pool(name="qp", bufs=2))
        epool = ctx.enter_context(tc.tile_pool(name="epool", bufs=4))
        # PSUM: ps_tr 2x[128,512](2 banks) ps_mm 3x[128,512](3) ps_s [2,1024](2)
        # ps_e 1x[128,512](1) -> 8 banks
        ps_tr = ctx.enter_context(tc.tile_pool(name="ps_tr", bufs=2, space="PSUM"))
        ps_mm = ctx.enter_context(tc.tile_pool(name="ps_mm", bufs=3, space="PSUM"))
        ps_s = ctx.enter_context(tc.tile_pool(name="ps_s", bufs=1, space="PSUM"))
        ps_e = ctx.enter_context(tc.tile_pool(name="ps_e", bufs=1, space="PSUM"))

        # ---------------- prologue ----------------
        identf = singles.tile([P, P], F32, tag="identf")
        make_identity(nc, identf)
        ident = singles.tile([P, P], F32, tag="ident")
        nc.scalar.copy(out=_r(ident), in_=identf)

        ones2b = singles.tile([P, 2], BF16, tag="ones2b")
        nc.gpsimd.memset(ones2b, 1.0)
        # [1,128] ones row, f32r-written: stationary for the K=1 E-broadcast
        onesrowf = singles.tile([1, P], F32, tag="onesrowf")
        nc.gpsimd.memset(onesrowf, 1.0)
        onesrow = singles.tile([1, P], F32, tag="onesrow")
        nc.scalar.copy(out=_r(onesrow), in_=onesrowf)

        a_flat = a_d.rearrange("f one -> (f one)")
        a1b = singles.tile([P, F], F32, tag="a1b")
        a2b = singles.tile([P, F], F32, tag="a2b")
        beta_sb = singles.tile([1, 1], F32, tag="beta_sb")
        w_sb = []
        wb = []
        for k in range(NK):
            wk = singles.tile([P, F], F32, tag=f"w_sb{k}")
            w_sb.append(wk)
            wbk = singles.tile([P, F], BF16, tag=f"wb{k}")
            wb.append(wbk)
        w12b = singles.tile([P, 2 * NK], BF16, tag="w12b")

        def load_weights():
            nc.sync.dma_start(out=a1b, in_=a_flat[0:F].partition_broadcast(P))
            nc.sync.dma_start(out=a2b, in_=a_flat[F : 2 * F].partition_broadcast(P))
            # beta lands in SBUF only to keep the input bound (value baked)
            nc.sync.dma_start(out=beta_sb, in_=beta_d[0:1].unsqueeze(0))
            for k in range(NK):
                nc.sync.dma_start(out=w_sb[k], in_=w_d[k * P : (k + 1) * P, :])
                # bf16 copy of W for the h matmul (moving operand)
                nc.scalar.copy(out=wb[k], in_=w_sb[k])
                w12f = qp.tile([P, 2], F32, tag="w12f")
                prod = qp.tile([P, F], F32, tag="wa_prod")
                for j, ab in enumerate((a1b, a2b)):
                    nc.vector.tensor_tensor(
                        out=prod, in0=w_sb[k], in1=ab, op=AL.mult
                    )
                    nc.vector.reduce_sum(
                        out=w12f[:, j : j + 1], in_=prod, axis=mybir.AxisListType.X
                    )
                nc.scalar.copy(out=w12b[:, 2 * k : 2 * k + 2], in_=w12f)

        # ---------------- PE warm-up ----------------
        # hold the activity monitor busy during the initial DMA window so
        # real matmuls start at the max clock
        for _ in range(10):
            wp = ps_tr.tile([P, F], F32, tag="ps_tr")
            nc.tensor.transpose(_r(wp[:, 0:P]), _r(ident), _r(ident))
            nc.tensor.transpose(_r(wp[:, P : 2 * P]), _r(ident), _r(ident))

        # ---------------- per-batch state ----------------
        xt_alls = {}
        h_sbs = {}
        uts = {}
        rcols = {}
        e1bs = {}
        e1abs = {}
        e2cols = {}
        e2acols = {}
        x_tiles = {}

        def phase_A_dma(b):  # issue all x loads for this batch
            x_tiles[b] = []
            for n in range(NN):
                x_t = xin.tile([P, F], F32, tag="x_t")
                nc.sync.dma_start(out=_r(x_t), in_=_r(x_d[b, n * P : (n + 1) * P, :]))
                x_tiles[b].append(x_t)

        def phase_T(b):  # transpose x into bf16 xT
            xt_all = xtp.tile([P, NK * N_NODES], BF16, tag="xt_all")
            xt_alls[b] = xt_all
            for n in range(NN):
                x_t = x_tiles[b][n]
                xp = ps_tr.tile([P, F], F32, tag="ps_tr")
                for k in range(NK):
                    nc.tensor.transpose(
                        _r(xp[:, k * P : (k + 1) * P]),
                        _r(x_t[:, k * P : (k + 1) * P]),
                        _r(ident),
                    )
                dst = xt_all.rearrange("p (k c) -> p k c", k=NK)[
                    :, :, n * P : (n + 1) * P
                ]
                src = xp.rearrange("p (k c) -> p k c", k=NK)
                # ACT: DVE is the scarce engine (C phase + epilogues)
                nc.scalar.copy(out=dst, in_=src)

        e1rows = {}
        e1arows = {}

        def phase_S(b):
            # s rows; u's factorization u = max(e^{s1}e^{s2},
            # e^{.2 s1}e^{.2 s2}) needs exps only on the s VECTORS, not on
            # the NxN matrix: E2/E2a as [128,8] columns (bias scalars),
            # E1/E1a as [1,N] rows that a K=1 PE matmul broadcasts to
            # [128,N] (a partition_broadcast DMA measured 12.5us; the PE
            # outer product is ~0.5us).
            xt_all = xt_alls[b]
            s_ps = ps_s.tile([2, N_NODES], F32, tag="ps_s")
            for k in range(NK):
                for hh in range(2):
                    nc.tensor.matmul(
                        s_ps[:, hh * F : (hh + 1) * F],
                        lhsT=w12b[:, 2 * k : 2 * k + 2],
                        rhs=xt_all[:, k * N_NODES + hh * F : k * N_NODES + (hh + 1) * F],
                        start=(k == 0),
                        stop=(k == NK - 1),
                    )
            s_sb = spool.tile([2, N_NODES], F32, tag="s_sb")
            nc.vector.tensor_copy(out=s_sb, in_=s_ps)
            # E1/E1a rows (f32r-written: they feed the broadcast matmul)
            e1row = spool.tile([1, N_NODES], F32, tag="e1row")
            e1rows[b] = e1row
            nc.scalar.activation(out=_r(e1row), in_=s_sb[0:1, :], func=AF.Exp)
            e1arow = spool.tile([1, N_NODES], F32, tag="e1arow")
            e1arows[b] = e1arow
            nc.scalar.activation(
                out=_r(e1arow), in_=s_sb[0:1, :], func=AF.Exp, scale=ALPHA
            )
            # s2 row -> per-partition columns through DRAM, then tiny exps
            nc.sync.dma_start(out=s_d[b].unsqueeze(0), in_=s_sb[1:2, :])
            s2col = spool.tile([P, NN], F32, tag="s2col")
            nc.sync.dma_start(out=s2col, in_=s_d[b].rearrange("(n p) -> p n", p=P))
            e2col = spool.tile([P, NN], F32, tag="e2col")
            nc.scalar.activation(out=e2col, in_=s2col, func=AF.Exp)
            e2cols[b] = e2col
            e2acol = spool.tile([P, NN], F32, tag="e2acol")
            nc.scalar.activation(out=e2acol, in_=s2col, func=AF.Exp, scale=ALPHA)
            e2acols[b] = e2acol

        def emit_E_bcast(b):
            # e1b/e1ab[p, i] = E1/E1a[i] via ones-column outer product;
            # bf16 SBUF copies so the C-phase tensor_scalars hit 4x_2p
            e1b = spool.tile([P, N_NODES], BF16, tag="e1b")
            e1bs[b] = e1b
            e1ab = spool.tile([P, N_NODES], BF16, tag="e1ab")
            e1abs[b] = e1ab
            for row, dstf in ((e1rows[b], e1b), (e1arows[b], e1ab)):
                for hh in range(2):
                    bp = ps_e.tile([P, F], F32, tag="ps_e")
                    nc.tensor.matmul(
                        bp,
                        lhsT=_r(onesrow),
                        rhs=_r(row[0:1, hh * F : (hh + 1) * F]),
                        start=True,
                        stop=True,
                    )
                    nc.scalar.copy(out=dstf[:, hh * F : (hh + 1) * F], in_=bp)

        def emit_C_tile(b, j):
            # uT[j][p, i] = max(E1[i]E2[jp], E1a[i]E2a[jp]) -- 3 DVE ops in
            # 2x fast mode (SBUF-only operands; bf16 outs make the max
            # 2x_1p-eligible; per-partition scalars are dtype-exempt).
            # Zero ACT, zero PE.
            t1 = tpool.tile([P, N_NODES], BF16, tag="t1")
            nc.vector.tensor_scalar(
                out=t1, in0=e1bs[b], scalar1=e2cols[b][:, j : j + 1], scalar2=None,
                op0=AL.mult,
            )
            t2 = tpool.tile([P, N_NODES], BF16, tag="t2")
            nc.vector.tensor_scalar(
                out=t2, in0=e1abs[b], scalar1=e2acols[b][:, j : j + 1], scalar2=None,
                op0=AL.mult,
            )
            u = utp.tile([P, N_NODES], BF16, tag="ut")
            nc.vector.tensor_tensor(out=u, in0=t1, in1=t2, op=AL.max)
            uts[b][j] = u

        def phase_C(b):
            uts[b] = [None] * NN
            for j in range(NN):
                emit_C_tile(b, j)

        def phase_B(b):  # h = x @ W  (bf16 out for the p matmul + epilogue)
            xt_all = xt_alls[b]
            h_sbs[b] = []
            for n in range(NN):
                h_ps = ps_mm.tile([P, F], F32, tag="ps_mm")
                for k in range(NK):
                    nc.tensor.matmul(
                        h_ps,
                        lhsT=xt_all[:, k * N_NODES + n * P : k * N_NODES + (n + 1) * P],
                        rhs=wb[k],
                        start=(k == 0),
                        stop=(k == NK - 1),
                    )
                ht = hpool.tile([P, F], BF16, tag="h_sb")
                nc.scalar.copy(out=ht, in_=h_ps)
                h_sbs[b].append(ht)
                if n == 2:
                    # E-broadcast matmuls slot in mid-B so the E rows (ACT)
                    # are ready and the C phase can start ~6 tiles early
                    emit_E_bcast(b)

        def phase_R(b):  # rowsum -> reciprocal columns
            ut = uts[b]
            rs_ps = ps_s.tile([2, N_NODES], F32, tag="ps_s")
            for j in range(NN):
                for hh in range(2):
                    nc.tensor.matmul(
                        rs_ps[:, hh * F : (hh + 1) * F],
                        lhsT=ones2b,
                        rhs=ut[j][:, hh * F : (hh + 1) * F],
                        start=(j == 0),
                        stop=(j == NN - 1),
                    )
            rrow = spool.tile([1, N_NODES], F32, tag="rrow")
            nc.vector.tensor_copy(out=rrow, in_=rs_ps[0:1, :])
            nc.sync.dma_start(out=r_d[b].unsqueeze(0), in_=rrow)
            rcraw = spool.tile([P, NN], F32, tag="rcraw")
            nc.sync.dma_start(out=rcraw, in_=r_d[b].rearrange("(n p) -> p n", p=P))
            rcol = spool.tile([P, NN], F32, tag="rcol")
            rcols[b] = rcol
            nc.vector.reciprocal(out=rcol, in_=rcraw)

        def emit_DE_tile(b, n):  # p[n] = u @ h + fused ELU epilogue
            ut, h_sb, rcol = uts[b], h_sbs[b], rcols[b]
            p_ps = ps_mm.tile([P, F], F32, tag="ps_mm")
            for j in range(NN):
                nc.tensor.matmul(
                    p_ps,
                    lhsT=ut[j][:, n * P : (n + 1) * P],
                    rhs=h_sb[j],
                    start=(j == 0),
                    stop=(j == NN - 1),
                )
            hin = h_sb[n]
            if beta_val != 1.0:
                hb = epool.tile([P, F], BF16, tag="hb")
                nc.vector.tensor_scalar_mul(hb, hin, float(beta_val))
                hin = hb
            # v = p*(1/rowsum) + beta*h; epilogue leans on DVE (ACT has the
            # copies), only the exp itself is ACT
            v = epool.tile([P, F], BF16, tag="v")
            nc.vector.scalar_tensor_tensor(
                out=v, in0=p_ps, scalar=rcol[:, n : n + 1], in1=hin,
                op0=AL.mult, op1=AL.add,
            )
            m = epool.tile([P, F], BF16, tag="m")
            nc.vector.tensor_scalar(
                out=m, in0=v, scalar1=0.0, scalar2=None, op0=AL.min
            )
            em = epool.tile([P, F], BF16, tag="em")
            nc.scalar.activation(out=em, in_=m, func=AF.Exp)
            o1 = epool.tile([P, F], BF16, tag="o1")
            nc.vector.tensor_scalar(
                out=o1, in0=em, scalar1=-1.0, scalar2=None, op0=AL.add
            )
            o = epool.tile([P, F], F32, tag="o")
            nc.vector.tensor_tensor(out=o, in0=o1, in1=v, op=AL.max)
            nc.sync.dma_start(out=out_d[b, n * P : (n + 1) * P, :], in_=o)

        # ------------- software-pipelined emission -------------
        # PE order: warmup T0 S0 B0 T1 S1 B1 R0 DE0 R1 DE1 -- back-to-back
        # matmuls, never paced by ACT/DVE.  C phases are pure ACT/DVE and
        # run concurrently (C0 under B0/T1, C1 under B1/DE0).
        phase_A_dma(0)
        load_weights()
        phase_T(0)
        phase_S(0)
        phase_A_dma(1)
        phase_B(0)
        phase_T(1)
        phase_C(0)
        phase_S(1)
        phase_B(1)
        phase_R(0)
        phase_C(1)
        for n in range(NN):
            if n == 5:
                phase_R(1)
            emit_DE_tile(0, n)
        for n in range(NN):
            emit_DE_tile(1, n)

    nc.finalize()
    return nc


_NC_CACHE = {}


def _get_nc(beta_val: float) -> bass.Bass:
    key = float(beta_val)
    if key not in _NC_CACHE:
        _NC_CACHE[key] = build_nc(beta_val=key)
    return _NC_CACHE[key]


def kernel(x, W, a, beta, _trace=False, _mm_fp32=False):
    x = np.ascontiguousarray(x, dtype=np.float32)
    W = np.ascontiguousarray(W, dtype=np.float32)
    a = np.ascontiguousarray(a, dtype=np.float32)
    beta = np.ascontiguousarray(beta, dtype=np.float32)

    nc = _get_nc(float(beta.reshape(-1)[0]))
    in_maps = [
        {
            "x": x[c * B_PER_CORE : (c + 1) * B_PER_CORE],
            "W": W,
            "a": a,
            "beta": beta,
        }
        for c in range(N_CORES)
    ]
    res = run_bass_kernel_spmd(nc, in_maps, core_ids=list(range(N_CORES)), trace=_trace)
    out = np.concatenate(
        [np.asarray(r["out"]).astype(np.float32) for r in res.results], axis=0
    )
    if _trace:
        kernel.last_exec_time_ns = res.exec_time_ns
        kernel.last_results = res
    return out


if __name__ == "__main__":
    rng = np.random.default_rng(0)
    x = rng.standard_normal((B_TOTAL, N_NODES, F), dtype=np.float32)
    W = rng.standard_normal((F, F), dtype=np.float32) * 0.05
    a = rng.standard_normal((2 * F, 1), dtype=np.float32) * 0.05
    beta = np.ones((1,), dtype=np.float32)
    out = kernel(x, W, a, beta)
    print("out", out.shape, out.dtype)


# revision 41
# speedup vs baseline: 1.3670x; 1.0391x over previous
"""Trainium2 Bass kernel for a batched GAT layer (BGATLayer).

Reference computation (per batch b of B=16, N=1024 nodes, F=512 features):
    h   = x @ W                                   # [N, F]
    s1  = h @ a1 ; s2 = h @ a2                    # [N]
    e   = leakyrelu(s1[:,None] + s2[None,:], 0.2) # [N, N]
    att = softmax(e, axis=1)                      # row softmax
    out = elu(att @ h + beta * h)                 # [N, F]

Sharding: batch B=16 split across 8 NeuronCores (2 batches/core, data
parallel); W/a/beta replicated.

v2 design (v1 measured 147us; PE busy 107us incl ~40us at HAM half
clock from C-phase starvation, plus a 19us epilogue tail):
  * All matmul operands are bf16 (tolerance is 2e-2; bf16 adds ~4e-3).
    Streaming rate is the same 1 cyc/row as f32r@512, but LDWEIGHTS is
    ~2x faster and SBUF footprint halves.
  * uT tiles are computed with ZERO PE work: uT[j][p,i] =
    exp(prelu(s1[i] + s2[j*128+p])).  s1 enters as a [128, N]
    partition-broadcast tile (SBUF->SBUF DMA), s2 as a per-partition
    bias column (ACT activation bias= accepts a [128,1] AP; DVE path
    uses tensor_scalar with an AP scalar).  v1 computed each z tile as
    a K=2 PE matmul that ping-ponged with ACT and starved the PE into
    the HAM's k=4/8 duty-cycle downclock.
  * PE stream is matmul-only and back-to-back:
      warmup T0 S0 B0 T1 S1 B1 R0 DE0(p) R1 DE1(p)
    so the activity monitor keeps the clock at max.
  * rowsum still via ones-stationary matmuls (cheap: 2-row stationary),
    reciprocal through the DRAM row->column roundtrip.
  * epilogue per tile: v = p*recip + h (DVE stt, mixed f32/bf16),
    m = min(v,0) (DVE), em = exp(m) (ACT), out = max(em-1, v) (DVE),
    trailing the p matmuls tile-by-tile instead of bunching at the end.
"""

import sys

sys.path.insert(0, "/opt/trn_rl_repo")

from contextlib import ExitStack

import numpy as np

import concourse.bacc as bacc
import concourse.bass as bass
import concourse.mybir as mybir
from concourse.bass_utils import run_bass_kernel_spmd
from concourse.masks import make_identity
from concourse.tile import TileContext

P = 128
N_NODES = 1024
F = 512
B_TOTAL = 16
N_CORES = 8
B_PER_CORE = B_TOTAL // N_CORES
NK = F // P  # 4 contraction chunks for x @ W
NN = N_NODES // P  # 8 node chunks
ALPHA = 0.2

F32 = mybir.dt.float32
F32R = mybir.dt.float32r
BF16 = mybir.dt.bfloat16
AL = mybir.AluOpType
AF = mybir.ActivationFunctionType


def _r(ap):
    """float32r view of an fp32 AP (PE reduced-precision matmul mode)."""
    return ap.bitcast(F32R)


def build_nc(beta_val: float = 1.0) -> bass.Bass:
    nc = bacc.Bacc("TRN2")
    x_d = nc.dram_tensor("x", [B_PER_CORE, N_NODES, F], F32, kind="ExternalInput")
    w_d = nc.dram_tensor("W", [F, F], F32, kind="ExternalInput")
    a_d = nc.dram_tensor("a", [2 * F, 1], F32, kind="ExternalInput")
    beta_d = nc.dram_tensor("beta", [1], F32, kind="ExternalInput")
    out_d = nc.dram_tensor("out", [B_PER_CORE, N_NODES, F], F32, kind="ExternalOutput")
    # scratch for row->per-partition-column roundtrips
    r_d = nc.dram_tensor("r_scratch", [B_PER_CORE, N_NODES], F32)
    s_d = nc.dram_tensor("s_scratch", [B_PER_CORE, N_NODES], F32)

    with TileContext(nc) as tc, ExitStack() as ctx:
        # ---------------- pools ----------------
        singles = ctx.enter_context(tc.tile_pool(name="singles", bufs=1))
        xin = ctx.enter_context(tc.tile_pool(name="xin", bufs=16))
        xtp = ctx.enter_context(tc.tile_pool(name="xtp", bufs=2))  # xT bf16
        hpool = ctx.enter_context(tc.tile_pool(name="hpool", bufs=16))
        spool = ctx.enter_context(tc.tile_pool(name="spool", bufs=2))
        utp = ctx.enter_context(tc.tile_pool(name="utp", bufs=16))
        tpool = ctx.enter_context(tc.tile_pool(name="tpool", bufs=3))
        qp = ctx.enter_context(tc.tile_pool(name="qp", bufs=2))
        epool = ctx.enter_context(tc.tile_pool(name="epool", bufs=4))
        # PSUM: ps_tr 2x[128,512](2 banks) ps_mm 3x[128,512](3) ps_s [2,1024](2)
        # ps_e 1x[128,512](1) -> 8 banks
        ps_tr = ctx.enter_context(tc.tile_pool(name="ps_tr", bufs=2, space="PSUM"))
        ps_mm = ctx.enter_context(tc.tile_pool(name="ps_mm", bufs=3, space="PSUM"))
        ps_s = ctx.enter_context(tc.tile_pool(name="ps_s", bufs=1, space="PSUM"))
        ps_e = ctx.enter_context(tc.tile_pool(name="ps_e", bufs=1, space="PSUM"))

        # ---------------- prologue ----------------
        identf = singles.tile([P, P], F32, tag="identf")
        make_identity(nc, identf)
        ident = singles.tile([P, P], F32, tag="ident")
        nc.scalar.copy(out=_r(ident), in_=identf)

        ones2b = singles.tile([P, 2], BF16, tag="ones2b")
        nc.gpsimd.memset(ones2b, 1.0)
        # [1,128] ones row, f32r-written: stationary for the K=1 E-broadcast
        onesrowf = singles.tile([1, P], F32, tag="onesrowf")
        nc.gpsimd.memset(onesrowf, 1.0)
        onesrow = singles.tile([1, P], F32, tag="onesrow")
        nc.scalar.copy(out=_r(onesrow), in_=onesrowf)

        a_flat = a_d.rearrange("f one -> (f one)")
        a1b = singles.tile([P, F], F32, tag="a1b")
        a2b = singles.tile([P, F], F32, tag="a2b")
        beta_sb = singles.tile([1, 1], F32, tag="beta_sb")
        w_sb = []
        wb = []
        for k in range(NK):
            wk = singles.tile([P, F], F32, tag=f"w_sb{k}")
            w_sb.append(wk)
            wbk = singles.tile([P, F], BF16, tag=f"wb{k}")
            wb.append(wbk)
        w12b = singles.tile([P, 2 * NK], BF16, tag="w12b")

        def load_weights():
            # weight DMAs ride the scalar engine's hardware queue so they
            # don't serialize behind the 4MB x stream on the sync queue
            nc.scalar.dma_start(out=a1b, in_=a_flat[0:F].partition_broadcast(P))
            nc.scalar.dma_start(out=a2b, in_=a_flat[F : 2 * F].partition_broadcast(P))
            # beta lands in SBUF only to keep the input bound (value baked)
            nc.scalar.dma_start(out=beta_sb, in_=beta_d[0:1].unsqueeze(0))
            for k in range(NK):
                nc.scalar.dma_start(out=w_sb[k], in_=w_d[k * P : (k + 1) * P, :])
                # bf16 copy of W for the h matmul (moving operand)
                nc.scalar.copy(out=wb[k], in_=w_sb[k])
                w12f = qp.tile([P, 2], F32, tag="w12f")
                prod = qp.tile([P, F], F32, tag="wa_prod")
                for j, ab in enumerate((a1b, a2b)):
                    # W@a via elementwise mult + per-partition accumulator
                    nc.vector.scalar_tensor_tensor(
                        out=prod, in0=w_sb[k], scalar=1.0, in1=ab,
                        op0=AL.mult, op1=AL.mult,
                        accum_out=w12f[:, j : j + 1],
                    )
                nc.scalar.copy(out=w12b[:, 2 * k : 2 * k + 2], in_=w12f)

        # ---------------- PE warm-up ----------------
        # hold the activity monitor busy during the initial DMA window so
        # real matmuls start at the max clock
        for _ in range(10):
            wp = ps_tr.tile([P, F], F32, tag="ps_tr")
            nc.tensor.transpose(_r(wp[:, 0:P]), _r(ident), _r(ident))
            nc.tensor.transpose(_r(wp[:, P : 2 * P]), _r(ident), _r(ident))

        # ---------------- per-batch state ----------------
        xt_alls = {}
        h_sbs = {}
        uts = {}
        rcols = {}
        e1bs = {}
        e1abs = {}
        e2cols = {}
        e2acols = {}
        x_tiles = {}

        def phase_A_dma(b):  # issue all x loads for this batch
            x_tiles[b] = []
            for n in range(NN):
                x_t = xin.tile([P, F], F32, tag="x_t")
                nc.sync.dma_start(out=_r(x_t), in_=_r(x_d[b, n * P : (n + 1) * P, :]))
                x_tiles[b].append(x_t)

        def phase_T(b):  # transpose x into bf16 xT
            xt_all = xtp.tile([P, NK * N_NODES], BF16, tag="xt_all")
            xt_alls[b] = xt_all
            for n in range(NN):
                x_t = x_tiles[b][n]
                xp = ps_tr.tile([P, F], F32, tag="ps_tr")
                for k in range(NK):
                    nc.tensor.transpose(
                        _r(xp[:, k * P : (k + 1) * P]),
                        _r(x_t[:, k * P : (k + 1) * P]),
                        _r(ident),
                    )
                dst = xt_all.rearrange("p (k c) -> p k c", k=NK)[
                    :, :, n * P : (n + 1) * P
                ]
                src = xp.rearrange("p (k c) -> p k c", k=NK)
                # alternate drains so neither busy engine paces the PE
                if n % 2 == 0:
                    nc.vector.tensor_copy(out=dst, in_=src)
                else:
                    nc.scalar.copy(out=dst, in_=src)

        e1rows = {}
        e1arows = {}

        def phase_S(b):
            # s rows; u's factorization u = max(e^{s1}e^{s2},
            # e^{.2 s1}e^{.2 s2}) needs exps only on the s VECTORS, not on
            # the NxN matrix: E2/E2a as [128,8] columns (bias scalars),
            # E1/E1a as [1,N] rows that a K=1 PE matmul broadcasts to
            # [128,N] (a partition_broadcast DMA measured 12.5us; the PE
            # outer product is ~0.5us).
            xt_all = xt_alls[b]
            s_ps = ps_s.tile([2, N_NODES], F32, tag="ps_s")
            for k in range(NK):
                for hh in range(2):
                    nc.tensor.matmul(
                        s_ps[:, hh * F : (hh + 1) * F],
                        lhsT=w12b[:, 2 * k : 2 * k + 2],
                        rhs=xt_all[:, k * N_NODES + hh * F : k * N_NODES + (hh + 1) * F],
                        start=(k == 0),
                        stop=(k == NK - 1),
                    )
            s_sb = spool.tile([2, N_NODES], F32, tag="s_sb")
            nc.vector.tensor_copy(out=s_sb, in_=s_ps)
            # E1/E1a rows (f32r-written: they feed the broadcast matmul)
            e1row = spool.tile([1, N_NODES], F32, tag="e1row")
            e1rows[b] = e1row
            nc.scalar.activation(out=_r(e1row), in_=s_sb[0:1, :], func=AF.Exp)
            e1arow = spool.tile([1, N_NODES], F32, tag="e1arow")
            e1arows[b] = e1arow
            nc.scalar.activation(
                out=_r(e1arow), in_=s_sb[0:1, :], func=AF.Exp, scale=ALPHA
            )
            # s2 row -> per-partition columns through DRAM, then tiny exps
            nc.sync.dma_start(out=s_d[b].unsqueeze(0), in_=s_sb[1:2, :])
            s2col = spool.tile([P, NN], F32, tag="s2col")
            nc.sync.dma_start(out=s2col, in_=s_d[b].rearrange("(n p) -> p n", p=P))
            e2col = spool.tile([P, NN], F32, tag="e2col")
            nc.scalar.activation(out=e2col, in_=s2col, func=AF.Exp)
            e2cols[b] = e2col
            e2acol = spool.tile([P, NN], F32, tag="e2acol")
            nc.scalar.activation(out=e2acol, in_=s2col, func=AF.Exp, scale=ALPHA)
            e2acols[b] = e2acol

        def emit_E_bcast(b):
            # e1b/e1ab[p, i] = E1/E1a[i] via ones-column outer product;
            # bf16 SBUF copies so the C-phase tensor_scalars hit 4x_2p
            e1b = spool.tile([P, N_NODES], BF16, tag="e1b")
            e1bs[b] = e1b
            e1ab = spool.tile([P, N_NODES], BF16, tag="e1ab")
            e1abs[b] = e1ab
            for row, dstf in ((e1rows[b], e1b), (e1arows[b], e1ab)):
                for hh in range(2):
                    bp = ps_e.tile([P, F], F32, tag="ps_e")
                    nc.tensor.matmul(
                        bp,
                        lhsT=_r(onesrow),
                        rhs=_r(row[0:1, hh * F : (hh + 1) * F]),
                        start=True,
                        stop=True,
                    )
                    nc.scalar.copy(out=dstf[:, hh * F : (hh + 1) * F], in_=bp)

        def emit_C_tile(b, j):
            # uT[j][p, i] = max(E1[i]E2[jp], E1a[i]E2a[jp]) -- 3 DVE ops in
            # 2x fast mode (SBUF-only operands; bf16 outs make the max
            # 2x_1p-eligible; per-partition scalars are dtype-exempt).
            # Zero ACT, zero PE.
            t1 = tpool.tile([P, N_NODES], BF16, tag="t1")
            nc.vector.tensor_scalar(
                out=t1, in0=e1bs[b], scalar1=e2cols[b][:, j : j + 1], scalar2=None,
                op0=AL.mult,
            )
            t2 = tpool.tile([P, N_NODES], BF16, tag="t2")
            nc.vector.tensor_scalar(
                out=t2, in0=e1abs[b], scalar1=e2acols[b][:, j : j + 1], scalar2=None,
                op0=AL.mult,
            )
            u = utp.tile([P, N_NODES], BF16, tag="ut")
            nc.vector.tensor_tensor(out=u, in0=t1, in1=t2, op=AL.max)
            uts[b][j] = u

        def phase_C(b):
            uts[b] = [None] * NN
            for j in range(NN):
                emit_C_tile(b, j)

        def phase_B(b):  # h = x @ W  (bf16 out for the p matmul + epilogue)
            xt_all = xt_alls[b]
            h_sbs[b] = []
            for n in range(NN):
                h_ps = ps_mm.tile([P, F], F32, tag="ps_mm")
                for k in range(NK):
                    nc.tensor.matmul(
                        h_ps,
                        lhsT=xt_all[:, k * N_NODES + n * P : k * N_NODES + (n + 1) * P],
                        rhs=wb[k],
                        start=(k == 0),
                        stop=(k == NK - 1),
                    )
                ht = hpool.tile([P, F], BF16, tag="h_sb")
                nc.scalar.copy(out=ht, in_=h_ps)
                h_sbs[b].append(ht)
                if n == 2:
                    # E-broadcast matmuls slot in mid-B so the E rows (ACT)
                    # are ready and the C phase can start ~6 tiles early
                    emit_E_bcast(b)

        def phase_R(b):  # rowsum -> reciprocal columns
            ut = uts[b]
            rs_ps = ps_s.tile([2, N_NODES], F32, tag="ps_s")
            for j in range(NN):
                for hh in range(2):
                    nc.tensor.matmul(
                        rs_ps[:, hh * F : (hh + 1) * F],
                        lhsT=ones2b,
                        rhs=ut[j][:, hh * F : (hh + 1) * F],
                        start=(j == 0),
                        stop=(j == NN - 1),
                    )
            rrow = spool.tile([1, N_NODES], F32, tag="rrow")
            nc.vector.tensor_copy(out=rrow, in_=rs_ps[0:1, :])
            nc.sync.dma_start(out=r_d[b].unsqueeze(0), in_=rrow)
            rcraw = spool.tile([P, NN], F32, tag="rcraw")
            nc.sync.dma_start(out=rcraw, in_=r_d[b].rearrange("(n p) -> p n", p=P))
            rcol = spool.tile([P, NN], F32, tag="rcol")
            rcols[b] = rcol
            nc.vector.reciprocal(out=rcol, in_=rcraw)

        def emit_DE_tile(b, n):  # p[n] = u @ h + fused ELU epilogue
            ut, h_sb, rcol = uts[b], h_sbs[b], rcols[b]
            p_ps = ps_mm.tile([P, F], F32, tag="ps_mm")
            for j in range(NN):
                nc.tensor.matmul(
                    p_ps,
                    lhsT=ut[j][:, n * P : (n + 1) * P],
                    rhs=h_sb[j],
                    start=(j == 0),
                    stop=(j == NN - 1),
                )
            hin = h_sb[n]
            if beta_val != 1.0:
                hb = epool.tile([P, F], BF16, tag="hb")
                nc.vector.tensor_scalar_mul(hb, hin, float(beta_val))
                hin = hb
            # v = p*(1/rowsum) + beta*h; epilogue leans on DVE (ACT has the
            # copies), only the exp itself is ACT
            v = epool.tile([P, F], BF16, tag="v")
            nc.vector.scalar_tensor_tensor(
                out=v, in0=p_ps, scalar=rcol[:, n : n + 1], in1=hin,
                op0=AL.mult, op1=AL.add,
            )
            m = epool.tile([P, F], BF16, tag="m")
            nc.vector.tensor_scalar(
                out=m, in0=v, scalar1=0.0, scalar2=None, op0=AL.min
            )
            em = epool.tile([P, F], BF16, tag="em")
            nc.scalar.activation(out=em, in_=m, func=AF.Exp)
            o1 = epool.tile([P, F], BF16, tag="o1")
            nc.vector.tensor_scalar(
                out=o1, in0=em, scalar1=-1.0, scalar2=None, op0=AL.add
            )
            o = epool.tile([P, F], F32, tag="o")
            nc.vector.tensor_tensor(out=o, in0=o1, in1=v, op=AL.max)
            nc.sync.dma_start(out=out_d[b, n * P : (n + 1) * P, :], in_=o)

        # ------------- software-pipelined emission -------------
        # PE order: warmup T0 S0 B0 T1 S1 B1 R0 DE0 R1 DE1 -- back-to-back
        # matmuls, never paced by ACT/DVE.  C phases are pure ACT/DVE and
        # run concurrently (C0 under B0/T1, C1 under B1/DE0).
        phase_A_dma(0)
        load_weights()
        phase_T(0)
        phase_S(0)
        phase_A_dma(1)
        phase_B(0)
        phase_T(1)
        phase_C(0)
        phase_S(1)
        phase_B(1)
        phase_R(0)
        phase_C(1)
        for n in range(NN):
            if n == 5:
                phase_R(1)
            emit_DE_tile(0, n)
        for n in range(NN):
            emit_DE_tile(1, n)

    nc.finalize()
    return nc


_NC_CACHE = {}


def _get_nc(beta_val: float) -> bass.Bass:
    key = float(beta_val)
    if key not in _NC_CACHE:
        _NC_CACHE[key] = build_nc(beta_val=key)
    return _NC_CACHE[key]


def kernel(x, W, a, beta, _trace=False, _mm_fp32=False):
    x = np.ascontiguousarray(x, dtype=np.float32)
    W = np.ascontiguousarray(W, dtype=np.float32)
    a = np.ascontiguousarray(a, dtype=np.float32)
    beta = np.ascontiguousarray(beta, dtype=np.float32)

    nc = _get_nc(float(beta.reshape(-1)[0]))
    in_maps = [
        {
            "x": x[c * B_PER_CORE : (c + 1) * B_PER_CORE],
            "W": W,
            "a": a,
            "beta": beta,
        }
        for c in range(N_CORES)
    ]
    res = run_bass_kernel_spmd(nc, in_maps, core_ids=list(range(N_CORES)), trace=_trace)
    out = np.concatenate(
        [np.asarray(r["out"]).astype(np.float32) for r in res.results], axis=0
    )
    if _trace:
        kernel.last_exec_time_ns = res.exec_time_ns
        kernel.last_results = res
    return out


if __name__ == "__main__":
    rng = np.random.default_rng(0)
    x = rng.standard_normal((B_TOTAL, N_NODES, F), dtype=np.float32)
    W = rng.standard_normal((F, F), dtype=np.float32) * 0.05
    a = rng.standard_normal((2 * F, 1), dtype=np.float32) * 0.05
    beta = np.ones((1,), dtype=np.float32)
    out = kernel(x, W, a, beta)
    print("out", out.shape, out.dtype)
